# revision 13
# baseline (speedup 1.0000x reference)
"""Trainium2 Bass kernel for nn_DistanceTransform.

The reference's data-dependent while-loop collapses to a closed form:
    d(p)   = Chebyshev distance from p to the nearest seed
    S(p)   = sum over the 3x3 neighborhood (replicate-clamped) of
             w(dy,dx) * [d(q) < d(p)]
    out(p) = 0 if d(p)==0 else (d(p)-1) - h*ln(S(p))

The Chebyshev DT decomposes exactly into four 1D min-plus passes:
    D* = diagNE(diagSE(seed0))          (cost 1 per step along diagonals)
    d  = min(axisX(D*), axisY(D*))      (cost 1 per step along rows/cols)
Each 1D pass is one forward+backward `tensor_tensor_scan` over all line
blocks concatenated in the free dim, with 256-wide INF separator regions
between blocks (a cross-block leak path costs >= 256 > max(d) = 255, so
leaks never win a min). Diagonal passes run in 45-degree-sheared layouts
produced by DRAM staging buffers with mismatched read/write row pitches;
reads come back through 16-bit DMA-transposes straight into the scan
layout. S(p) uses PE banded matmuls for row-shifted d and DVE is_lt
masks.

Data-parallel over B*C = 2 images: core b computes image b.
"""

import os
import numpy as np

import concourse.bacc as bacc
import concourse.mybir as mybir
from concourse.tile import TileContext
from concourse.masks import make_identity
from concourse.bass_utils import run_bass_kernel_spmd

F32 = mybir.dt.float32
F16 = mybir.dt.float16
I16 = mybir.dt.int16
AF = mybir.ActivationFunctionType
ALU = mybir.AluOpType

H = W = 256
HB = 2
INF = 1536.0
H_PARAM = np.float32(0.35)
E1 = float(np.exp(np.float32(-1.0) / H_PARAM))
EC = float(np.exp(np.float32(-np.sqrt(np.float32(2.0))) / H_PARAM))
LNSCALE = float(np.exp(np.float32(1.0) / H_PARAM))

P1R = 516   # stage1 read pitch (f16); write pitch 515, base 255: c = x+255-y
P2R = 768   # stage2 read pitch (f16); write pitch 770: c' = c+2y-255
P3R = 516   # stage3 read pitch (f16); write pitch 515: x = c'-y

N_CORES = 8


def _build_program():
    nc = bacc.Bacc("TRN2", target_bir_lowering=False, debug=False,
                   num_devices=N_CORES)
    img = nc.dram_tensor("img", [H, W], F32, kind="ExternalInput").ap()
    out = nc.dram_tensor("out", [H, W], F32, kind="ExternalOutput").ap()
    stage1 = nc.dram_tensor("stage1", [256 * P1R + 600], F16).ap()
    stage2 = nc.dram_tensor("stage2", [256 * P2R + 1200], F16).ap()
    stage3 = nc.dram_tensor("stage3", [256 * P3R + 600], F16).ap()

    dbg = {}
    if os.environ.get("DT_DEBUG"):
        for name, shape in [("dbg_d", [H, W]), ("dbg_dstar", [H, W]),
                            ("dbg_s", [H, W])]:
            dbg[name] = nc.dram_tensor(name, shape, F32,
                                       kind="ExternalOutput").ap()

    with TileContext(nc) as tc:
        _emit(nc, tc, img, out, stage1, stage2, stage3, dbg)
    nc.compile()
    return nc


def _emit(nc, tc, img, out, stage1, stage2, stage3, dbg=None):
    import contextlib
    dbg = dbg or {}
    ctx = contextlib.ExitStack()
    const = ctx.enter_context(tc.tile_pool(name="const", bufs=1))
    work = ctx.enter_context(tc.tile_pool(name="work", bufs=1))
    psum = ctx.enter_context(tc.tile_pool(name="psum", bufs=4, space="PSUM"))
    psum2 = ctx.enter_context(tc.tile_pool(name="psum2", bufs=2, space="PSUM"))

    # ---------------- constants / init ------------------------------------
    ident16 = const.tile([128, 128], F16)
    make_identity(nc, ident16[:])

    ones16 = const.tile([128, 2048], F16)
    nc.gpsimd.memset(ones16[:], 1.0)

    zero16 = const.tile([128, 1032], F16)
    nc.gpsimd.memset(zero16[:], 0.0)

    zero32 = const.tile([128, 512], F32)
    nc.gpsimd.memset(zero32[:], 0.0)

    inf16 = const.tile([128, 1540], F16)
    nc.gpsimd.memset(inf16[:], INF)

    # prefill stage1 with zeros (maps to INF after the seed transform)
    n1 = 256 * P1R
    nc.sync.dma_start(
        out=stage1[:n1].rearrange("(p f) -> p f", p=128),
        in_=zero16[:, : n1 // 128])
    # prefill stage2 with INF
    n2 = 256 * P2R
    nc.sync.dma_start(
        out=stage2[:n2].rearrange("(p f) -> p f", p=128),
        in_=inf16[:, : n2 // 128])

    # sanitize mask (transposed layout): MBT[p, cb, y] = 1 where
    # c' = 128*cb + p is outside [y, y+255]. For cb in {0,1} only c'-y < 0
    # can be invalid; for cb in {2,3} only c'-y > 255.
    VT = const.tile([128, 4, 256], F16)
    nc.vector.memset(VT[:], 1.0)
    for cb in range(4):
        if cb < 2:
            nc.gpsimd.affine_select(   # valid iff (128*cb + p) - y >= 0
                out=VT[:, cb, :], in_=VT[:, cb, :], compare_op=ALU.is_ge,
                fill=0.0, base=128 * cb, pattern=[[-1, 256]],
                channel_multiplier=1)
        else:
            nc.gpsimd.affine_select(   # valid iff 255 - (128*cb + p) + y >= 0
                out=VT[:, cb, :], in_=VT[:, cb, :], compare_op=ALU.is_ge,
                fill=0.0, base=255 - 128 * cb, pattern=[[1, 256]],
                channel_multiplier=-1)
    FILLT = const.tile([128, 4, 256], F16)   # (1 - V) * INF
    nc.scalar.activation(FILLT[:], VT[:], AF.Copy, bias=INF, scale=-INF)

    # banded matrices for row shifts (lhsT: [k, m] = weight of in-row k in
    # out-row m). up: out[m] = in[m-1] (replicate top); down: out[m]=in[m+1].
    def band(tile_ap, diag_base, corner=None):
        nc.gpsimd.memset(tile_ap, 0.0)
        nc.gpsimd.affine_select(
            out=tile_ap, in_=tile_ap, compare_op=ALU.not_equal, fill=1.0,
            base=diag_base, pattern=[[-1, 128]], channel_multiplier=1)
        if corner == "tl":
            nc.gpsimd.affine_select(
                out=tile_ap, in_=tile_ap, compare_op=ALU.not_equal, fill=1.0,
                base=0, pattern=[[1, 128]], channel_multiplier=1)
        elif corner == "br":
            nc.gpsimd.affine_select(
                out=tile_ap, in_=tile_ap, compare_op=ALU.not_equal, fill=1.0,
                base=-254, pattern=[[1, 128]], channel_multiplier=1)

    supA = const.tile([128, 128], F16)
    band(supA[:], 1, corner="tl")
    supB = const.tile([128, 128], F16)
    band(supB[:], 1)
    sdnA = const.tile([128, 128], F16)
    band(sdnA[:], -1)
    sdnB = const.tile([128, 128], F16)
    band(sdnB[:], -1, corner="br")
    e_up = const.tile([128, 128], F16)   # 1 at [k=127, m=0]
    nc.gpsimd.memset(e_up[:], 0.0)
    nc.gpsimd.affine_select(
        out=e_up[:], in_=e_up[:], compare_op=ALU.not_equal, fill=1.0,
        base=127, pattern=[[1, 128]], channel_multiplier=-1)
    e_dn = const.tile([128, 128], F16)   # 1 at [k=0, m=127]
    nc.gpsimd.memset(e_dn[:], 0.0)
    nc.gpsimd.affine_select(
        out=e_dn[:], in_=e_dn[:], compare_op=ALU.not_equal, fill=1.0,
        base=127, pattern=[[-1, 128]], channel_multiplier=1)

    # seed image for the final stage (off critical path)
    img32 = work.tile([128, HB, 256], F32)
    for b in range(HB):
        nc.sync.dma_start(out=img32[:, b, :], in_=img[128 * b:128 * (b + 1), :])
    inv32 = work.tile([128, HB, 256], F32)   # 1 - seed
    nc.scalar.activation(inv32[:], img32[:], AF.Copy, bias=1.0, scale=-1.0)

    # ---------------- shear A + transpose-in ------------------------------
    # cast band write f32->f16: row y at 255 + 515*y + x; read c = x+255-y
    bandA = stage1[255: 255 + 256 * (P1R - 1)].rearrange(
        "(y f) -> y f", f=P1R - 1)[:, :W]
    nc.gpsimd.dma_start(out=bandA, in_=img[:, :])

    d1raw = work.tile([128, 4, 512], F16)
    for cb in range(4):
        rd = stage1[: 256 * P1R].rearrange(
            "(y f) -> y f", f=P1R)[:, 128 * cb:128 * (cb + 1)]
        nc.sync.dma_start(out=d1raw[:, cb, :256], in_=rd, transpose=True)

    d1 = work.tile([128, 4, 512], F16)
    nc.scalar.activation(d1[:, :, :256], d1raw[:, :, :256], AF.Copy,
                         bias=INF, scale=-INF)
    nc.vector.memset(d1[:, :, 256:], INF)

    # ---------------- merged min-plus pass helper --------------------------
    def minplus(arr, tmp_tag):
        nblk = arr.shape[1]
        flat = arr[:].rearrange("p a b -> p (a b)")
        tmp = work.tile([128, nblk, 512], F16, tag=tmp_tag)
        tflat = tmp[:].rearrange("p a b -> p (a b)")
        nc.vector.tensor_tensor_scan(
            out=tflat, data0=ones16[:, :512 * nblk], data1=flat,
            initial=INF, op0=ALU.add, op1=ALU.min)
        nc.vector.memset(tmp[:, :, 256:], INF)
        nc.vector.tensor_tensor_scan(
            out=flat[:, ::-1], data0=ones16[:, :512 * nblk],
            data1=tflat[:, ::-1], initial=INF, op0=ALU.add, op1=ALU.min)

    minplus(d1, "scan1")          # diag SE pass (lines c = x-y+255)

    # ---------------- transpose-back + shear B -----------------------------
    def transpose_4to2(src, dst):
        k = 0
        for yb in range(2):
            for cb in range(4):
                pt = psum.tile([128, 128], F16, tag="tp")
                nc.tensor.transpose(
                    pt[:], src[:, cb, 128 * yb:128 * (yb + 1)], ident16[:])
                if k % 2 == 0:
                    nc.scalar.copy(out=dst[:, yb, 128 * cb:128 * (cb + 1)],
                                   in_=pt[:])
                else:
                    nc.vector.tensor_copy(
                        out=dst[:, yb, 128 * cb:128 * (cb + 1)], in_=pt[:])
                k += 1

    sk1b = work.tile([128, HB, 512], F16)
    transpose_4to2(d1, sk1b)

    # write(y, c) at 770*y + c ; read(y, c') at 768*y + 255 + c'
    for b in range(HB):
        wr = stage2[128 * b * (P2R + 2): (128 * b + 128) * (P2R + 2)].rearrange(
            "(y f) -> y f", f=P2R + 2)[:, :512]
        nc.sync.dma_start(out=wr, in_=sk1b[:, b, :])
    d2 = work.tile([128, 4, 512], F16)
    for cb in range(4):
        rd = stage2[255: 255 + 256 * P2R].rearrange(
            "(y f) -> y f", f=P2R)[:, 128 * cb:128 * (cb + 1)]
        nc.sync.dma_start(out=d2[:, cb, :256], in_=rd, transpose=True)
    nc.vector.memset(d2[:, :, 256:], INF)
    nc.vector.tensor_tensor(out=d2[:, :, :256], in0=d2[:, :, :256],
                            in1=VT[:], op=ALU.mult)
    nc.vector.tensor_tensor(out=d2[:, :, :256], in0=d2[:, :, :256],
                            in1=FILLT[:], op=ALU.add)

    minplus(d2, "scan2")          # diag NE pass (lines c' = x+y)

    sk2b = work.tile([128, HB, 512], F16)
    transpose_4to2(d2, sk2b)

    # ---------------- unshear C -------------------------------------------
    # write(y, c') at 515*y + c' ; read(y, x) at 516*y + x  (x = c'-y)
    for b in range(HB):
        wr = stage3[128 * b * (P3R - 1): (128 * b + 128) * (P3R - 1)].rearrange(
            "(y f) -> y f", f=P3R - 1)[:, :512]
        nc.sync.dma_start(out=wr, in_=sk2b[:, b, :])
    dstar = work.tile([128, HB, 512], F16)
    for b in range(HB):
        rd = stage3[128 * b * P3R: (128 * b + 128) * P3R].rearrange(
            "(y f) -> y f", f=P3R)[:, :256]
        nc.sync.dma_start(out=dstar[:, b, :256], in_=rd)
    nc.vector.memset(dstar[:, :, 256:], INF)
    dstT = work.tile([128, HB, 512], F16)
    for xb in range(2):
        rd = stage3[: 256 * P3R].rearrange(
            "(y f) -> y f", f=P3R)[:, 128 * xb:128 * (xb + 1)]
        nc.sync.dma_start(out=dstT[:, xb, :256], in_=rd, transpose=True)
    nc.vector.memset(dstT[:, :, 256:], INF)

    if "dbg_dstar" in dbg:
        t = work.tile([128, HB, 256], F32, tag="dbg1")
        nc.vector.tensor_copy(out=t[:], in_=dstar[:, :, :256])
        for b in range(HB):
            nc.sync.dma_start(out=dbg["dbg_dstar"][128 * b:128 * (b + 1), :],
                              in_=t[:, b, :])

    # ---------------- axis passes ------------------------------------------
    minplus(dstar, "scan3")       # axisX in image layout
    minplus(dstT, "scan4")        # axisY in transposed layout
    dy = work.tile([128, HB, 256], F16)
    k = 0
    for yb in range(2):
        for xb in range(2):
            pt = psum.tile([128, 128], F16, tag="tp")
            nc.tensor.transpose(
                pt[:], dstT[:, xb, 128 * yb:128 * (yb + 1)], ident16[:])
            if k % 2 == 0:
                nc.scalar.copy(out=dy[:, yb, 128 * xb:128 * (xb + 1)],
                               in_=pt[:])
            else:
                nc.vector.tensor_copy(out=dy[:, yb, 128 * xb:128 * (xb + 1)],
                                      in_=pt[:])
            k += 1

    d16 = work.tile([128, HB, 256], F16)
    nc.vector.tensor_tensor(out=d16[:], in0=dstar[:, :, :256], in1=dy[:],
                            op=ALU.min)

    if "dbg_d" in dbg:
        t = work.tile([128, HB, 256], F32, tag="dbg2")
        nc.vector.tensor_copy(out=t[:], in_=d16[:])
        for b in range(HB):
            nc.sync.dma_start(out=dbg["dbg_d"][128 * b:128 * (b + 1), :],
                              in_=t[:, b, :])

    # ---------------- S stage ---------------------------------------------
    pup = psum2.tile([128, HB, 256], F32, tag="pup")
    nc.tensor.matmul(pup[:, 0, :], supA[:], d16[:, 0, :], start=True, stop=True)
    nc.tensor.matmul(pup[:, 1, :], supB[:], d16[:, 1, :], start=True, stop=False)
    nc.tensor.matmul(pup[:, 1, :], e_up[:], d16[:, 0, :], start=False, stop=True)
    pdn = psum2.tile([128, HB, 256], F32, tag="pdn")
    nc.tensor.matmul(pdn[:, 0, :], sdnA[:], d16[:, 0, :], start=True, stop=False)
    nc.tensor.matmul(pdn[:, 0, :], e_dn[:], d16[:, 1, :], start=False, stop=True)
    nc.tensor.matmul(pdn[:, 1, :], sdnB[:], d16[:, 1, :], start=True, stop=True)
    up16 = work.tile([128, HB, 256], F16)
    nc.scalar.copy(out=up16[:], in_=pup[:])
    dn16 = work.tile([128, HB, 256], F16)
    nc.scalar.copy(out=dn16[:], in_=pdn[:])

    GA = work.tile([128, 4, HB, 256], F16)
    GD = work.tile([128, 4, HB, 256], F16)
    for b in range(HB):   # zero only the never-written border columns
        nc.gpsimd.memset(GA[:, 2, b, 0:1], 0.0)
        nc.gpsimd.memset(GA[:, 3, b, 255:256], 0.0)

    nc.vector.tensor_tensor(out=GA[:, 0], in0=up16[:], in1=d16[:], op=ALU.is_lt)
    nc.vector.tensor_tensor(out=GA[:, 1], in0=dn16[:], in1=d16[:], op=ALU.is_lt)
    for b in range(HB):
        nc.vector.tensor_tensor(
            out=GA[:, 2, b, 1:], in0=d16[:, b, :-1], in1=d16[:, b, 1:],
            op=ALU.is_lt)
        nc.vector.tensor_tensor(
            out=GA[:, 3, b, :-1], in0=d16[:, b, 1:], in1=d16[:, b, :-1],
            op=ALU.is_lt)
        nc.vector.tensor_tensor(
            out=GD[:, 0, b, 1:], in0=up16[:, b, :-1], in1=d16[:, b, 1:],
            op=ALU.is_lt)
        nc.vector.tensor_tensor(
            out=GD[:, 1, b, :-1], in0=up16[:, b, 1:], in1=d16[:, b, :-1],
            op=ALU.is_lt)
        nc.vector.tensor_tensor(
            out=GD[:, 2, b, 1:], in0=dn16[:, b, :-1], in1=d16[:, b, 1:],
            op=ALU.is_lt)
        nc.vector.tensor_tensor(
            out=GD[:, 3, b, :-1], in0=dn16[:, b, 1:], in1=d16[:, b, :-1],
            op=ALU.is_lt)
    for b in range(HB):   # x-border clamp: diagonals collapse onto verticals
        nc.scalar.copy(out=GD[:, 0, b, 0:1], in_=GA[:, 0, b, 0:1])
        nc.scalar.copy(out=GD[:, 2, b, 0:1], in_=GA[:, 1, b, 0:1])
        nc.scalar.copy(out=GD[:, 1, b, 255:256], in_=GA[:, 0, b, 255:256])
        nc.scalar.copy(out=GD[:, 3, b, 255:256], in_=GA[:, 1, b, 255:256])

    sa01 = work.tile([128, HB, 256], F16)
    nc.vector.tensor_tensor(out=sa01[:], in0=GA[:, 0], in1=GA[:, 1], op=ALU.add)
    sa23 = work.tile([128, HB, 256], F16)
    nc.vector.tensor_tensor(out=sa23[:], in0=GA[:, 2], in1=GA[:, 3], op=ALU.add)
    SA = work.tile([128, HB, 256], F16)
    nc.vector.tensor_tensor(out=SA[:], in0=sa01[:], in1=sa23[:], op=ALU.add)
    sd01 = work.tile([128, HB, 256], F16)
    nc.vector.tensor_tensor(out=sd01[:], in0=GD[:, 0], in1=GD[:, 1], op=ALU.add)
    sd23 = work.tile([128, HB, 256], F16)
    nc.vector.tensor_tensor(out=sd23[:], in0=GD[:, 2], in1=GD[:, 3], op=ALU.add)
    SD = work.tile([128, HB, 256], F16)
    nc.vector.tensor_tensor(out=SD[:], in0=sd01[:], in1=sd23[:], op=ALU.add)

    sa32 = work.tile([128, HB, 256], F32)
    nc.scalar.activation(sa32[:], SA[:], AF.Copy, bias=0.0, scale=E1)
    s32 = work.tile([128, HB, 256], F32)
    nc.vector.scalar_tensor_tensor(
        out=s32[:], in0=SD[:], scalar=EC, in1=sa32[:],
        op0=ALU.mult, op1=ALU.add)

    if "dbg_s" in dbg:
        for b in range(HB):
            nc.sync.dma_start(out=dbg["dbg_s"][128 * b:128 * (b + 1), :],
                              in_=s32[:, b, :])

    sg = work.tile([128, HB, 256], F32)
    nc.vector.tensor_tensor(out=sg[:], in0=s32[:], in1=img32[:], op=ALU.add)
    lnv = work.tile([128, HB, 256], F32)
    nc.scalar.activation(lnv[:], sg[:], AF.Ln, bias=0.0, scale=LNSCALE)
    outp = work.tile([128, HB, 256], F32)
    nc.vector.scalar_tensor_tensor(
        out=outp[:], in0=lnv[:], scalar=float(-H_PARAM), in1=d16[:],
        op0=ALU.mult, op1=ALU.add)
    nc.vector.tensor_tensor(out=outp[:], in0=outp[:], in1=inv32[:],
                            op=ALU.mult)

    for b in range(HB):
        nc.sync.dma_start(out=out[128 * b:128 * (b + 1), :], in_=outp[:, b, :])

    ctx.close()


_NC_CACHE = None


def _get_nc():
    global _NC_CACHE
    if _NC_CACHE is None:
        _NC_CACHE = _build_program()
    return _NC_CACHE


def kernel(image: np.ndarray) -> np.ndarray:
    """image: (2, 1, 256, 256) float32 -> (2, 1, 256, 256) float32."""
    B, C, Himg, Wimg = image.shape
    flat = np.ascontiguousarray(
        image.reshape(B * C, Himg, Wimg).astype(np.float32))
    n_units = flat.shape[0]
    nc = _get_nc()
    in_maps = [{"img": flat[i % n_units]} for i in range(N_CORES)]
    res = run_bass_kernel_spmd(nc, in_maps, core_ids=list(range(N_CORES)))
    outs = [res.results[i]["out"] for i in range(n_units)]
    return np.stack(outs).reshape(B, C, Himg, Wimg).astype(image.dtype)


if __name__ == "__main__":
    from concourse.bass_interp import CoreSim
    import jax
    cpu = jax.devices("cpu")[0]
    with jax.default_device(cpu):
        import reference as R
        inputs = R.setup_inputs()
        img_np = np.asarray(inputs["image"]).reshape(2, 256, 256)
        expected = np.asarray(R.reference(**inputs)).reshape(2, 256, 256)
    print("reference done", flush=True)
    nc = _get_nc()
    print("program built", flush=True)
    sim = CoreSim(nc)
    sim.tensor("img")[:] = img_np[0]
    sim.simulate()
    got = sim.tensor("out").copy()
    err = np.abs(got - expected[0])
    rel = err.max() / (np.abs(expected[0]).max() + 1e-9)
    print("sim image0: max abs err", err.max(), "rel", rel)


# revision 14
# speedup vs baseline: 1.2566x; 1.2566x over previous
"""Trainium2 Bass kernel for nn_DistanceTransform.

The reference's data-dependent while-loop collapses to a closed form:
    d(p)   = Chebyshev distance from p to the nearest seed
    S(p)   = sum over the 3x3 neighborhood (replicate-clamped) of
             w(dy,dx) * [d(q) < d(p)]
    out(p) = 0 if d(p)==0 else (d(p)-1) - h*ln(S(p))

The Chebyshev DT decomposes exactly into four 1D min-plus passes:
    D* = diagNE(diagSE(seed0))          (cost 1 per step along diagonals)
    d  = min(axisX(D*), axisY(D*))      (cost 1 per step along rows/cols)
Each 1D pass is one forward+backward `tensor_tensor_scan` over all line
blocks concatenated in the free dim, with 256-wide INF separator regions
between blocks (a cross-block leak path costs >= 256 > max(d) = 255, so
leaks never win a min). Diagonal passes run in 45-degree-sheared layouts
produced by DRAM staging buffers with mismatched read/write row pitches;
reads come back through 16-bit DMA-transposes straight into the scan
layout. S(p) uses PE banded matmuls for row-shifted d and DVE is_lt
masks.

Data-parallel over B*C = 2 images: core b computes image b.
"""

import os
import numpy as np

import concourse.bacc as bacc
import concourse.mybir as mybir
from concourse.tile import TileContext
from concourse.masks import make_identity
from concourse.bass_utils import run_bass_kernel_spmd

F32 = mybir.dt.float32
F16 = mybir.dt.float16
I16 = mybir.dt.int16
AF = mybir.ActivationFunctionType
ALU = mybir.AluOpType

H = W = 256
HB = 2
INF = 1536.0
H_PARAM = np.float32(0.35)
E1 = float(np.exp(np.float32(-1.0) / H_PARAM))
EC = float(np.exp(np.float32(-np.sqrt(np.float32(2.0))) / H_PARAM))
LNSCALE = float(np.exp(np.float32(1.0) / H_PARAM))

P1R = 516   # stage1 read pitch (f16); write pitch 515, base 255: c = x+255-y
P2R = 768   # stage2 read pitch (f16); write pitch 770: c' = c+2y-255
P3R = 516   # stage3 read pitch (f16); write pitch 515: x = c'-y

N_CORES = 8


def _build_program():
    nc = bacc.Bacc("TRN2", target_bir_lowering=False, debug=False,
                   num_devices=N_CORES)
    img = nc.dram_tensor("img", [H, W], F32, kind="ExternalInput").ap()
    out = nc.dram_tensor("out", [H, W], F32, kind="ExternalOutput").ap()
    stage1 = nc.dram_tensor("stage1", [256 * P1R + 600], F16).ap()
    stage2 = nc.dram_tensor("stage2", [256 * P2R + 1200], F16).ap()
    stage3 = nc.dram_tensor("stage3", [256 * P3R + 600], F16).ap()

    dbg = {}
    if os.environ.get("DT_DEBUG"):
        for name, shape in [("dbg_d", [H, W]), ("dbg_dstar", [H, W]),
                            ("dbg_s", [H, W])]:
            dbg[name] = nc.dram_tensor(name, shape, F32,
                                       kind="ExternalOutput").ap()

    with TileContext(nc) as tc:
        _emit(nc, tc, img, out, stage1, stage2, stage3, dbg)
    nc.compile()
    return nc


def _emit(nc, tc, img, out, stage1, stage2, stage3, dbg=None):
    import contextlib
    dbg = dbg or {}
    ctx = contextlib.ExitStack()
    const = ctx.enter_context(tc.tile_pool(name="const", bufs=1))
    work = ctx.enter_context(tc.tile_pool(name="work", bufs=1))
    psum = ctx.enter_context(tc.tile_pool(name="psum", bufs=4, space="PSUM"))
    psum2 = ctx.enter_context(tc.tile_pool(name="psum2", bufs=2, space="PSUM"))

    # ---------------- constants / init ------------------------------------
    ident16 = const.tile([128, 128], F16)
    make_identity(nc, ident16[:])

    ones16 = const.tile([128, 256], F16)
    nc.gpsimd.memset(ones16[:], 1.0)

    zero16 = const.tile([128, 1032], F16)
    nc.gpsimd.memset(zero16[:], 0.0)

    zero32 = const.tile([128, 512], F32)
    nc.gpsimd.memset(zero32[:], 0.0)

    inf16 = const.tile([128, 1540], F16)
    nc.gpsimd.memset(inf16[:], INF)

    # prefill stage1 with zeros (maps to INF after the seed transform)
    n1 = 256 * P1R
    nc.sync.dma_start(
        out=stage1[:n1].rearrange("(p f) -> p f", p=128),
        in_=zero16[:, : n1 // 128])
    # prefill stage2 with INF
    n2 = 256 * P2R
    nc.sync.dma_start(
        out=stage2[:n2].rearrange("(p f) -> p f", p=128),
        in_=inf16[:, : n2 // 128])

    # sanitize mask (transposed layout): MBT[p, cb, y] = 1 where
    # c' = 128*cb + p is outside [y, y+255]. For cb in {0,1} only c'-y < 0
    # can be invalid; for cb in {2,3} only c'-y > 255.
    VT = const.tile([128, 4, 256], F16)
    nc.vector.memset(VT[:], 1.0)
    for cb in range(4):
        if cb < 2:
            nc.gpsimd.affine_select(   # valid iff (128*cb + p) - y >= 0
                out=VT[:, cb, :], in_=VT[:, cb, :], compare_op=ALU.is_ge,
                fill=0.0, base=128 * cb, pattern=[[-1, 256]],
                channel_multiplier=1)
        else:
            nc.gpsimd.affine_select(   # valid iff 255 - (128*cb + p) + y >= 0
                out=VT[:, cb, :], in_=VT[:, cb, :], compare_op=ALU.is_ge,
                fill=0.0, base=255 - 128 * cb, pattern=[[1, 256]],
                channel_multiplier=-1)
    FILLT = const.tile([128, 4, 256], F16)   # (1 - V) * INF
    nc.scalar.activation(FILLT[:], VT[:], AF.Copy, bias=INF, scale=-INF)

    # banded matrices for row shifts (lhsT: [k, m] = weight of in-row k in
    # out-row m). up: out[m] = in[m-1] (replicate top); down: out[m]=in[m+1].
    def band(tile_ap, diag_base, corner=None):
        nc.gpsimd.memset(tile_ap, 0.0)
        nc.gpsimd.affine_select(
            out=tile_ap, in_=tile_ap, compare_op=ALU.not_equal, fill=1.0,
            base=diag_base, pattern=[[-1, 128]], channel_multiplier=1)
        if corner == "tl":
            nc.gpsimd.affine_select(
                out=tile_ap, in_=tile_ap, compare_op=ALU.not_equal, fill=1.0,
                base=0, pattern=[[1, 128]], channel_multiplier=1)
        elif corner == "br":
            nc.gpsimd.affine_select(
                out=tile_ap, in_=tile_ap, compare_op=ALU.not_equal, fill=1.0,
                base=-254, pattern=[[1, 128]], channel_multiplier=1)

    supA = const.tile([128, 128], F16)
    band(supA[:], 1, corner="tl")
    supB = const.tile([128, 128], F16)
    band(supB[:], 1)
    sdnA = const.tile([128, 128], F16)
    band(sdnA[:], -1)
    sdnB = const.tile([128, 128], F16)
    band(sdnB[:], -1, corner="br")
    e_up = const.tile([128, 128], F16)   # 1 at [k=127, m=0]
    nc.gpsimd.memset(e_up[:], 0.0)
    nc.gpsimd.affine_select(
        out=e_up[:], in_=e_up[:], compare_op=ALU.not_equal, fill=1.0,
        base=127, pattern=[[1, 128]], channel_multiplier=-1)
    e_dn = const.tile([128, 128], F16)   # 1 at [k=0, m=127]
    nc.gpsimd.memset(e_dn[:], 0.0)
    nc.gpsimd.affine_select(
        out=e_dn[:], in_=e_dn[:], compare_op=ALU.not_equal, fill=1.0,
        base=127, pattern=[[-1, 128]], channel_multiplier=1)

    # seed image for the final stage (off critical path)
    img32 = work.tile([128, HB, 256], F32)
    for b in range(HB):
        nc.sync.dma_start(out=img32[:, b, :], in_=img[128 * b:128 * (b + 1), :])
    inv32 = work.tile([128, HB, 256], F32)   # 1 - seed
    nc.scalar.activation(inv32[:], img32[:], AF.Copy, bias=1.0, scale=-1.0)

    # ---------------- shear A + transpose-in ------------------------------
    # cast band write f32->f16: row y at 255 + 515*y + x; read c = x+255-y
    bandA = stage1[255: 255 + 256 * (P1R - 1)].rearrange(
        "(y f) -> y f", f=P1R - 1)[:, :W]
    nc.gpsimd.dma_start(out=bandA, in_=img[:, :])

    d1raw = work.tile([128, 4, 256], F16)
    for cb in range(4):
        rd = stage1[: 256 * P1R].rearrange(
            "(y f) -> y f", f=P1R)[:, 128 * cb:128 * (cb + 1)]
        eng = nc.sync if cb % 2 == 0 else nc.scalar
        eng.dma_start(out=d1raw[:, cb, :], in_=rd, transpose=True)

    d1 = work.tile([128, 4, 256], F16)
    nc.scalar.activation(d1[:], d1raw[:], AF.Copy, bias=INF, scale=-INF)

    # ---------------- per-block min-plus pass helper ------------------------
    def minplus(arr, tmp_tag):
        nblk = arr.shape[1]
        for cb in range(nblk):
            tmp = work.tile([128, 256], F16, tag=tmp_tag)
            nc.vector.tensor_tensor_scan(
                out=tmp[:], data0=ones16[:], data1=arr[:, cb, :],
                initial=INF, op0=ALU.add, op1=ALU.min)
            nc.vector.tensor_tensor_scan(
                out=arr[:, cb, ::-1], data0=ones16[:],
                data1=tmp[:, ::-1], initial=INF, op0=ALU.add, op1=ALU.min)

    minplus(d1, "scan1")          # diag SE pass (lines c = x-y+255)

    # ---------------- transpose-back + shear B -----------------------------
    def transpose_4to2(src, dst):
        k = 0
        for yb in range(2):
            for cb in range(4):
                pt = psum.tile([128, 128], F16, tag="tp")
                nc.tensor.transpose(
                    pt[:], src[:, cb, 128 * yb:128 * (yb + 1)], ident16[:])
                if k % 2 == 0:
                    nc.scalar.copy(out=dst[:, yb, 128 * cb:128 * (cb + 1)],
                                   in_=pt[:])
                else:
                    nc.vector.tensor_copy(
                        out=dst[:, yb, 128 * cb:128 * (cb + 1)], in_=pt[:])
                k += 1

    sk1b = work.tile([128, HB, 512], F16)
    transpose_4to2(d1, sk1b)

    # write(y, c) at 770*y + c ; read(y, c') at 768*y + 255 + c'
    for b in range(HB):
        wr = stage2[128 * b * (P2R + 2): (128 * b + 128) * (P2R + 2)].rearrange(
            "(y f) -> y f", f=P2R + 2)[:, :512]
        nc.sync.dma_start(out=wr, in_=sk1b[:, b, :])
    d2 = work.tile([128, 4, 256], F16)
    for cb in range(4):
        rd = stage2[255: 255 + 256 * P2R].rearrange(
            "(y f) -> y f", f=P2R)[:, 128 * cb:128 * (cb + 1)]
        eng = nc.sync if cb % 2 == 0 else nc.scalar
        eng.dma_start(out=d2[:, cb, :], in_=rd, transpose=True)
    nc.vector.tensor_tensor(out=d2[:], in0=d2[:], in1=VT[:], op=ALU.mult)
    nc.vector.tensor_tensor(out=d2[:], in0=d2[:], in1=FILLT[:], op=ALU.add)

    minplus(d2, "scan2")          # diag NE pass (lines c' = x+y)

    sk2b = work.tile([128, HB, 512], F16)
    transpose_4to2(d2, sk2b)

    # ---------------- unshear C -------------------------------------------
    # write(y, c') at 515*y + c' ; read(y, x) at 516*y + x  (x = c'-y)
    for b in range(HB):
        wr = stage3[128 * b * (P3R - 1): (128 * b + 128) * (P3R - 1)].rearrange(
            "(y f) -> y f", f=P3R - 1)[:, :512]
        nc.sync.dma_start(out=wr, in_=sk2b[:, b, :])
    dstar = work.tile([128, HB, 256], F16)
    for b in range(HB):
        rd = stage3[128 * b * P3R: (128 * b + 128) * P3R].rearrange(
            "(y f) -> y f", f=P3R)[:, :256]
        nc.sync.dma_start(out=dstar[:, b, :], in_=rd)
    dstT = work.tile([128, HB, 256], F16)
    for xb in range(2):
        rd = stage3[: 256 * P3R].rearrange(
            "(y f) -> y f", f=P3R)[:, 128 * xb:128 * (xb + 1)]
        nc.scalar.dma_start(out=dstT[:, xb, :], in_=rd, transpose=True)

    if "dbg_dstar" in dbg:
        t = work.tile([128, HB, 256], F32, tag="dbg1")
        nc.vector.tensor_copy(out=t[:], in_=dstar[:])
        for b in range(HB):
            nc.sync.dma_start(out=dbg["dbg_dstar"][128 * b:128 * (b + 1), :],
                              in_=t[:, b, :])

    # ---------------- axis passes ------------------------------------------
    minplus(dstar, "scan3")       # axisX in image layout
    minplus(dstT, "scan4")        # axisY in transposed layout
    dy = work.tile([128, HB, 256], F16)
    k = 0
    for yb in range(2):
        for xb in range(2):
            pt = psum.tile([128, 128], F16, tag="tp")
            nc.tensor.transpose(
                pt[:], dstT[:, xb, 128 * yb:128 * (yb + 1)], ident16[:])
            if k % 2 == 0:
                nc.scalar.copy(out=dy[:, yb, 128 * xb:128 * (xb + 1)],
                               in_=pt[:])
            else:
                nc.vector.tensor_copy(out=dy[:, yb, 128 * xb:128 * (xb + 1)],
                                      in_=pt[:])
            k += 1

    d16 = work.tile([128, HB, 256], F16)
    nc.vector.tensor_tensor(out=d16[:], in0=dstar[:], in1=dy[:], op=ALU.min)

    if "dbg_d" in dbg:
        t = work.tile([128, HB, 256], F32, tag="dbg2")
        nc.vector.tensor_copy(out=t[:], in_=d16[:])
        for b in range(HB):
            nc.sync.dma_start(out=dbg["dbg_d"][128 * b:128 * (b + 1), :],
                              in_=t[:, b, :])

    # ---------------- S stage ---------------------------------------------
    pup = psum2.tile([128, HB, 256], F32, tag="pup")
    nc.tensor.matmul(pup[:, 0, :], supA[:], d16[:, 0, :], start=True, stop=True)
    nc.tensor.matmul(pup[:, 1, :], supB[:], d16[:, 1, :], start=True, stop=False)
    nc.tensor.matmul(pup[:, 1, :], e_up[:], d16[:, 0, :], start=False, stop=True)
    pdn = psum2.tile([128, HB, 256], F32, tag="pdn")
    nc.tensor.matmul(pdn[:, 0, :], sdnA[:], d16[:, 0, :], start=True, stop=False)
    nc.tensor.matmul(pdn[:, 0, :], e_dn[:], d16[:, 1, :], start=False, stop=True)
    nc.tensor.matmul(pdn[:, 1, :], sdnB[:], d16[:, 1, :], start=True, stop=True)
    up16 = work.tile([128, HB, 256], F16)
    nc.scalar.copy(out=up16[:], in_=pup[:])
    dn16 = work.tile([128, HB, 256], F16)
    nc.scalar.copy(out=dn16[:], in_=pdn[:])

    GA = work.tile([128, 4, HB, 256], F16)
    GD = work.tile([128, 4, HB, 256], F16)
    for b in range(HB):   # zero only the never-written border columns
        nc.gpsimd.memset(GA[:, 2, b, 0:1], 0.0)
        nc.gpsimd.memset(GA[:, 3, b, 255:256], 0.0)

    nc.vector.tensor_tensor(out=GA[:, 0], in0=up16[:], in1=d16[:], op=ALU.is_lt)
    nc.vector.tensor_tensor(out=GA[:, 1], in0=dn16[:], in1=d16[:], op=ALU.is_lt)
    for b in range(HB):
        nc.vector.tensor_tensor(
            out=GA[:, 2, b, 1:], in0=d16[:, b, :-1], in1=d16[:, b, 1:],
            op=ALU.is_lt)
        nc.vector.tensor_tensor(
            out=GA[:, 3, b, :-1], in0=d16[:, b, 1:], in1=d16[:, b, :-1],
            op=ALU.is_lt)
        nc.vector.tensor_tensor(
            out=GD[:, 0, b, 1:], in0=up16[:, b, :-1], in1=d16[:, b, 1:],
            op=ALU.is_lt)
        nc.vector.tensor_tensor(
            out=GD[:, 1, b, :-1], in0=up16[:, b, 1:], in1=d16[:, b, :-1],
            op=ALU.is_lt)
        nc.vector.tensor_tensor(
            out=GD[:, 2, b, 1:], in0=dn16[:, b, :-1], in1=d16[:, b, 1:],
            op=ALU.is_lt)
        nc.vector.tensor_tensor(
            out=GD[:, 3, b, :-1], in0=dn16[:, b, 1:], in1=d16[:, b, :-1],
            op=ALU.is_lt)
    for b in range(HB):   # x-border clamp: diagonals collapse onto verticals
        nc.scalar.copy(out=GD[:, 0, b, 0:1], in_=GA[:, 0, b, 0:1])
        nc.scalar.copy(out=GD[:, 2, b, 0:1], in_=GA[:, 1, b, 0:1])
        nc.scalar.copy(out=GD[:, 1, b, 255:256], in_=GA[:, 0, b, 255:256])
        nc.scalar.copy(out=GD[:, 3, b, 255:256], in_=GA[:, 1, b, 255:256])

    sa01 = work.tile([128, HB, 256], F16)
    nc.vector.tensor_tensor(out=sa01[:], in0=GA[:, 0], in1=GA[:, 1], op=ALU.add)
    sa23 = work.tile([128, HB, 256], F16)
    nc.vector.tensor_tensor(out=sa23[:], in0=GA[:, 2], in1=GA[:, 3], op=ALU.add)
    SA = work.tile([128, HB, 256], F16)
    nc.vector.tensor_tensor(out=SA[:], in0=sa01[:], in1=sa23[:], op=ALU.add)
    sd01 = work.tile([128, HB, 256], F16)
    nc.vector.tensor_tensor(out=sd01[:], in0=GD[:, 0], in1=GD[:, 1], op=ALU.add)
    sd23 = work.tile([128, HB, 256], F16)
    nc.vector.tensor_tensor(out=sd23[:], in0=GD[:, 2], in1=GD[:, 3], op=ALU.add)
    SD = work.tile([128, HB, 256], F16)
    nc.vector.tensor_tensor(out=SD[:], in0=sd01[:], in1=sd23[:], op=ALU.add)

    sa32 = work.tile([128, HB, 256], F32)
    nc.scalar.activation(sa32[:], SA[:], AF.Copy, bias=0.0, scale=E1)
    s32 = work.tile([128, HB, 256], F32)
    nc.vector.scalar_tensor_tensor(
        out=s32[:], in0=SD[:], scalar=EC, in1=sa32[:],
        op0=ALU.mult, op1=ALU.add)

    if "dbg_s" in dbg:
        for b in range(HB):
            nc.sync.dma_start(out=dbg["dbg_s"][128 * b:128 * (b + 1), :],
                              in_=s32[:, b, :])

    sg = work.tile([128, HB, 256], F32)
    nc.vector.tensor_tensor(out=sg[:], in0=s32[:], in1=img32[:], op=ALU.add)
    lnv = work.tile([128, HB, 256], F32)
    nc.scalar.activation(lnv[:], sg[:], AF.Ln, bias=0.0, scale=LNSCALE)
    outp = work.tile([128, HB, 256], F32)
    nc.vector.scalar_tensor_tensor(
        out=outp[:], in0=lnv[:], scalar=float(-H_PARAM), in1=d16[:],
        op0=ALU.mult, op1=ALU.add)
    nc.vector.tensor_tensor(out=outp[:], in0=outp[:], in1=inv32[:],
                            op=ALU.mult)

    for b in range(HB):
        nc.sync.dma_start(out=out[128 * b:128 * (b + 1), :], in_=outp[:, b, :])

    ctx.close()


_NC_CACHE = None


def _get_nc():
    global _NC_CACHE
    if _NC_CACHE is None:
        _NC_CACHE = _build_program()
    return _NC_CACHE


def kernel(image: np.ndarray) -> np.ndarray:
    """image: (2, 1, 256, 256) float32 -> (2, 1, 256, 256) float32."""
    B, C, Himg, Wimg = image.shape
    flat = np.ascontiguousarray(
        image.reshape(B * C, Himg, Wimg).astype(np.float32))
    n_units = flat.shape[0]
    nc = _get_nc()
    in_maps = [{"img": flat[i % n_units]} for i in range(N_CORES)]
    res = run_bass_kernel_spmd(nc, in_maps, core_ids=list(range(N_CORES)))
    outs = [res.results[i]["out"] for i in range(n_units)]
    return np.stack(outs).reshape(B, C, Himg, Wimg).astype(image.dtype)


if __name__ == "__main__":
    from concourse.bass_interp import CoreSim
    import jax
    cpu = jax.devices("cpu")[0]
    with jax.default_device(cpu):
        import reference as R
        inputs = R.setup_inputs()
        img_np = np.asarray(inputs["image"]).reshape(2, 256, 256)
        expected = np.asarray(R.reference(**inputs)).reshape(2, 256, 256)
    print("reference done", flush=True)
    nc = _get_nc()
    print("program built", flush=True)
    sim = CoreSim(nc)
    sim.tensor("img")[:] = img_np[0]
    sim.simulate()
    got = sim.tensor("out").copy()
    err = np.abs(got - expected[0])
    rel = err.max() / (np.abs(expected[0]).max() + 1e-9)
    print("sim image0: max abs err", err.max(), "rel", rel)


# revision 15
# speedup vs baseline: 1.3252x; 1.0546x over previous
"""Trainium2 Bass kernel for nn_DistanceTransform.

The reference's data-dependent while-loop collapses to a closed form:
    d(p)   = Chebyshev distance from p to the nearest seed
    S(p)   = sum over the 3x3 neighborhood (replicate-clamped) of
             w(dy,dx) * [d(q) < d(p)]
    out(p) = 0 if d(p)==0 else (d(p)-1) - h*ln(S(p))

The Chebyshev DT decomposes exactly into four 1D min-plus passes:
    D* = diagNE(diagSE(seed0))          (cost 1 per step along diagonals)
    d  = min(axisX(D*), axisY(D*))      (cost 1 per step along rows/cols)
Each 1D pass is one forward+backward `tensor_tensor_scan` over all line
blocks concatenated in the free dim, with 256-wide INF separator regions
between blocks (a cross-block leak path costs >= 256 > max(d) = 255, so
leaks never win a min). Diagonal passes run in 45-degree-sheared layouts
produced by DRAM staging buffers with mismatched read/write row pitches;
reads come back through 16-bit DMA-transposes straight into the scan
layout. S(p) uses PE banded matmuls for row-shifted d and DVE is_lt
masks.

Data-parallel over B*C = 2 images: core b computes image b.
"""

import os
import numpy as np

import concourse.bacc as bacc
import concourse.mybir as mybir
from concourse.tile import TileContext
from concourse.masks import make_identity
from concourse.bass_utils import run_bass_kernel_spmd

F32 = mybir.dt.float32
F16 = mybir.dt.float16
I16 = mybir.dt.int16
AF = mybir.ActivationFunctionType
ALU = mybir.AluOpType

H = W = 256
HB = 2
INF = 1536.0
H_PARAM = np.float32(0.35)
E1 = float(np.exp(np.float32(-1.0) / H_PARAM))
EC = float(np.exp(np.float32(-np.sqrt(np.float32(2.0))) / H_PARAM))
LNSCALE = float(np.exp(np.float32(1.0) / H_PARAM))

P1R = 516   # stage1 read pitch (f16); write pitch 515, base 255: c = x+255-y
P2R = 768   # stage2 read pitch (f16); write pitch 770: c' = c+2y-255
P3R = 516   # stage3 read pitch (f16); write pitch 515: x = c'-y

N_CORES = 8


def _build_program():
    nc = bacc.Bacc("TRN2", target_bir_lowering=False, debug=False,
                   num_devices=N_CORES)
    img = nc.dram_tensor("img", [H, W], F32, kind="ExternalInput").ap()
    out = nc.dram_tensor("out", [H, W], F32, kind="ExternalOutput").ap()
    stage1 = nc.dram_tensor("stage1", [256 * P1R + 600], F16).ap()
    stage2 = nc.dram_tensor("stage2", [256 * P2R + 1200], F16).ap()
    stage3 = nc.dram_tensor("stage3", [256 * P3R + 600], F16).ap()

    dbg = {}
    if os.environ.get("DT_DEBUG"):
        for name, shape in [("dbg_d", [H, W]), ("dbg_dstar", [H, W]),
                            ("dbg_s", [H, W])]:
            dbg[name] = nc.dram_tensor(name, shape, F32,
                                       kind="ExternalOutput").ap()

    with TileContext(nc) as tc:
        _emit(nc, tc, img, out, stage1, stage2, stage3, dbg)
    nc.compile()
    return nc


def _emit(nc, tc, img, out, stage1, stage2, stage3, dbg=None):
    import contextlib
    dbg = dbg or {}
    ctx = contextlib.ExitStack()
    const = ctx.enter_context(tc.tile_pool(name="const", bufs=1))
    work = ctx.enter_context(tc.tile_pool(name="work", bufs=1))
    psum = ctx.enter_context(tc.tile_pool(name="psum", bufs=4, space="PSUM"))
    psum2 = ctx.enter_context(tc.tile_pool(name="psum2", bufs=2, space="PSUM"))

    # ---------------- critical-path head: prefills + sheared input --------
    zero16 = const.tile([128, 1032], F16)
    nc.gpsimd.memset(zero16[:], 0.0)
    inf16 = const.tile([128, 1540], F16)
    nc.vector.memset(inf16[:], INF)

    # prefill stage1 with zeros (maps to INF after the seed transform)
    n1 = 256 * P1R
    nc.sync.dma_start(
        out=stage1[:n1].rearrange("(p f) -> p f", p=128),
        in_=zero16[:, : n1 // 128])
    # prefill stage2 with INF
    n2 = 256 * P2R
    nc.scalar.dma_start(
        out=stage2[:n2].rearrange("(p f) -> p f", p=128),
        in_=inf16[:, : n2 // 128])
    # cast band write f32->f16: row y at 255 + 515*y + x; read c = x+255-y
    bandA0 = stage1[255: 255 + 256 * (P1R - 1)].rearrange(
        "(y f) -> y f", f=P1R - 1)[:, :W]
    nc.gpsimd.dma_start(out=bandA0, in_=img[:, :])

    # ---------------- constants / init ------------------------------------
    ident16 = const.tile([128, 128], F16)
    make_identity(nc, ident16[:])

    ones16 = const.tile([128, 256], F16)
    nc.gpsimd.memset(ones16[:], 1.0)

    zero32 = const.tile([128, 512], F32)
    nc.gpsimd.memset(zero32[:], 0.0)

    # sanitize mask (transposed layout): MBT[p, cb, y] = 1 where
    # c' = 128*cb + p is outside [y, y+255]. For cb in {0,1} only c'-y < 0
    # can be invalid; for cb in {2,3} only c'-y > 255.
    VT = const.tile([128, 4, 256], F16)
    nc.vector.memset(VT[:], 1.0)
    for cb in range(4):
        if cb < 2:
            nc.gpsimd.affine_select(   # valid iff (128*cb + p) - y >= 0
                out=VT[:, cb, :], in_=VT[:, cb, :], compare_op=ALU.is_ge,
                fill=0.0, base=128 * cb, pattern=[[-1, 256]],
                channel_multiplier=1)
        else:
            nc.gpsimd.affine_select(   # valid iff 255 - (128*cb + p) + y >= 0
                out=VT[:, cb, :], in_=VT[:, cb, :], compare_op=ALU.is_ge,
                fill=0.0, base=255 - 128 * cb, pattern=[[1, 256]],
                channel_multiplier=-1)
    FILLT = const.tile([128, 4, 256], F16)   # (1 - V) * INF
    nc.scalar.activation(FILLT[:], VT[:], AF.Copy, bias=INF, scale=-INF)

    # banded matrices for row shifts (lhsT: [k, m] = weight of in-row k in
    # out-row m). up: out[m] = in[m-1] (replicate top); down: out[m]=in[m+1].
    def band(tile_ap, diag_base, corner=None):
        nc.gpsimd.memset(tile_ap, 0.0)
        nc.gpsimd.affine_select(
            out=tile_ap, in_=tile_ap, compare_op=ALU.not_equal, fill=1.0,
            base=diag_base, pattern=[[-1, 128]], channel_multiplier=1)
        if corner == "tl":
            nc.gpsimd.affine_select(
                out=tile_ap, in_=tile_ap, compare_op=ALU.not_equal, fill=1.0,
                base=0, pattern=[[1, 128]], channel_multiplier=1)
        elif corner == "br":
            nc.gpsimd.affine_select(
                out=tile_ap, in_=tile_ap, compare_op=ALU.not_equal, fill=1.0,
                base=-254, pattern=[[1, 128]], channel_multiplier=1)

    supA = const.tile([128, 128], F16)
    band(supA[:], 1, corner="tl")
    supB = const.tile([128, 128], F16)
    band(supB[:], 1)
    sdnA = const.tile([128, 128], F16)
    band(sdnA[:], -1)
    sdnB = const.tile([128, 128], F16)
    band(sdnB[:], -1, corner="br")
    e_up = const.tile([128, 128], F16)   # 1 at [k=127, m=0]
    nc.gpsimd.memset(e_up[:], 0.0)
    nc.gpsimd.affine_select(
        out=e_up[:], in_=e_up[:], compare_op=ALU.not_equal, fill=1.0,
        base=127, pattern=[[1, 128]], channel_multiplier=-1)
    e_dn = const.tile([128, 128], F16)   # 1 at [k=0, m=127]
    nc.gpsimd.memset(e_dn[:], 0.0)
    nc.gpsimd.affine_select(
        out=e_dn[:], in_=e_dn[:], compare_op=ALU.not_equal, fill=1.0,
        base=127, pattern=[[-1, 128]], channel_multiplier=1)

    # seed image for the final stage (off critical path)
    img32 = work.tile([128, HB, 256], F32)
    for b in range(HB):
        nc.sync.dma_start(out=img32[:, b, :], in_=img[128 * b:128 * (b + 1), :])
    inv32 = work.tile([128, HB, 256], F32)   # 1 - seed
    nc.scalar.activation(inv32[:], img32[:], AF.Copy, bias=1.0, scale=-1.0)

    # ---------------- shear A + transpose-in ------------------------------
    d1raw = work.tile([128, 4, 256], F16)
    for cb in range(4):
        rd = stage1[: 256 * P1R].rearrange(
            "(y f) -> y f", f=P1R)[:, 128 * cb:128 * (cb + 1)]
        eng = nc.sync if cb % 2 == 0 else nc.scalar
        eng.dma_start(out=d1raw[:, cb, :], in_=rd, transpose=True)

    d1 = work.tile([128, 4, 256], F16)
    nc.scalar.activation(d1[:], d1raw[:], AF.Copy, bias=INF, scale=-INF)

    # ---------------- per-block min-plus pass helper ------------------------
    def minplus(arr, tmp_tag):
        nblk = arr.shape[1]
        for cb in range(nblk):
            tmp = work.tile([128, 256], F16, tag=tmp_tag)
            nc.vector.tensor_tensor_scan(
                out=tmp[:], data0=ones16[:], data1=arr[:, cb, :],
                initial=INF, op0=ALU.add, op1=ALU.min)
            nc.vector.tensor_tensor_scan(
                out=arr[:, cb, ::-1], data0=ones16[:],
                data1=tmp[:, ::-1], initial=INF, op0=ALU.add, op1=ALU.min)

    minplus(d1, "scan1")          # diag SE pass (lines c = x-y+255)

    # ---------------- transpose-back + shear B -----------------------------
    def transpose_4to2(src, dst):
        k = 0
        for yb in range(2):
            for cb in range(4):
                pt = psum.tile([128, 128], F16, tag="tp")
                nc.tensor.transpose(
                    pt[:], src[:, cb, 128 * yb:128 * (yb + 1)], ident16[:])
                if k % 2 == 0:
                    nc.scalar.copy(out=dst[:, yb, 128 * cb:128 * (cb + 1)],
                                   in_=pt[:])
                else:
                    nc.vector.tensor_copy(
                        out=dst[:, yb, 128 * cb:128 * (cb + 1)], in_=pt[:])
                k += 1

    sk1b = work.tile([128, HB, 512], F16)
    transpose_4to2(d1, sk1b)

    # write(y, c) at 770*y + c ; read(y, c') at 768*y + 255 + c'
    for b in range(HB):
        wr = stage2[128 * b * (P2R + 2): (128 * b + 128) * (P2R + 2)].rearrange(
            "(y f) -> y f", f=P2R + 2)[:, :512]
        nc.sync.dma_start(out=wr, in_=sk1b[:, b, :])
    d2 = work.tile([128, 4, 256], F16)
    for cb in range(4):
        rd = stage2[255: 255 + 256 * P2R].rearrange(
            "(y f) -> y f", f=P2R)[:, 128 * cb:128 * (cb + 1)]
        eng = nc.sync if cb % 2 == 0 else nc.scalar
        eng.dma_start(out=d2[:, cb, :], in_=rd, transpose=True)
    nc.vector.tensor_tensor(out=d2[:], in0=d2[:], in1=VT[:], op=ALU.mult)
    nc.vector.tensor_tensor(out=d2[:], in0=d2[:], in1=FILLT[:], op=ALU.add)

    minplus(d2, "scan2")          # diag NE pass (lines c' = x+y)

    sk2b = work.tile([128, HB, 512], F16)
    transpose_4to2(d2, sk2b)

    # ---------------- unshear C -------------------------------------------
    # write(y, c') at 515*y + c' ; read(y, x) at 516*y + x  (x = c'-y)
    for b in range(HB):
        wr = stage3[128 * b * (P3R - 1): (128 * b + 128) * (P3R - 1)].rearrange(
            "(y f) -> y f", f=P3R - 1)[:, :512]
        nc.sync.dma_start(out=wr, in_=sk2b[:, b, :])
    dstar = work.tile([128, HB, 256], F16)
    for b in range(HB):
        rd = stage3[128 * b * P3R: (128 * b + 128) * P3R].rearrange(
            "(y f) -> y f", f=P3R)[:, :256]
        nc.sync.dma_start(out=dstar[:, b, :], in_=rd)
    dstT = work.tile([128, HB, 256], F16)
    for xb in range(2):
        rd = stage3[: 256 * P3R].rearrange(
            "(y f) -> y f", f=P3R)[:, 128 * xb:128 * (xb + 1)]
        nc.scalar.dma_start(out=dstT[:, xb, :], in_=rd, transpose=True)

    if "dbg_dstar" in dbg:
        t = work.tile([128, HB, 256], F32, tag="dbg1")
        nc.vector.tensor_copy(out=t[:], in_=dstar[:])
        for b in range(HB):
            nc.sync.dma_start(out=dbg["dbg_dstar"][128 * b:128 * (b + 1), :],
                              in_=t[:, b, :])

    # ---------------- axis passes ------------------------------------------
    minplus(dstar, "scan3")       # axisX in image layout
    minplus(dstT, "scan4")        # axisY in transposed layout
    dy = work.tile([128, HB, 256], F16)
    k = 0
    for yb in range(2):
        for xb in range(2):
            pt = psum.tile([128, 128], F16, tag="tp")
            nc.tensor.transpose(
                pt[:], dstT[:, xb, 128 * yb:128 * (yb + 1)], ident16[:])
            if k % 2 == 0:
                nc.scalar.copy(out=dy[:, yb, 128 * xb:128 * (xb + 1)],
                               in_=pt[:])
            else:
                nc.vector.tensor_copy(out=dy[:, yb, 128 * xb:128 * (xb + 1)],
                                      in_=pt[:])
            k += 1

    d16 = work.tile([128, HB, 256], F16)
    nc.vector.tensor_tensor(out=d16[:], in0=dstar[:], in1=dy[:], op=ALU.min)

    if "dbg_d" in dbg:
        t = work.tile([128, HB, 256], F32, tag="dbg2")
        nc.vector.tensor_copy(out=t[:], in_=d16[:])
        for b in range(HB):
            nc.sync.dma_start(out=dbg["dbg_d"][128 * b:128 * (b + 1), :],
                              in_=t[:, b, :])

    # ---------------- S stage ---------------------------------------------
    pup = psum2.tile([128, HB, 256], F32, tag="pup")
    nc.tensor.matmul(pup[:, 0, :], supA[:], d16[:, 0, :], start=True, stop=True)
    nc.tensor.matmul(pup[:, 1, :], supB[:], d16[:, 1, :], start=True, stop=False)
    nc.tensor.matmul(pup[:, 1, :], e_up[:], d16[:, 0, :], start=False, stop=True)
    pdn = psum2.tile([128, HB, 256], F32, tag="pdn")
    nc.tensor.matmul(pdn[:, 0, :], sdnA[:], d16[:, 0, :], start=True, stop=False)
    nc.tensor.matmul(pdn[:, 0, :], e_dn[:], d16[:, 1, :], start=False, stop=True)
    nc.tensor.matmul(pdn[:, 1, :], sdnB[:], d16[:, 1, :], start=True, stop=True)
    up16 = work.tile([128, HB, 256], F16)
    nc.scalar.copy(out=up16[:], in_=pup[:])
    dn16 = work.tile([128, HB, 256], F16)
    nc.scalar.copy(out=dn16[:], in_=pdn[:])

    GA = work.tile([128, 4, HB, 256], F16)
    GD = work.tile([128, 4, HB, 256], F16)
    for b in range(HB):   # zero only the never-written border columns
        nc.gpsimd.memset(GA[:, 2, b, 0:1], 0.0)
        nc.gpsimd.memset(GA[:, 3, b, 255:256], 0.0)

    nc.vector.tensor_tensor(out=GA[:, 0], in0=up16[:], in1=d16[:], op=ALU.is_lt)
    nc.vector.tensor_tensor(out=GA[:, 1], in0=dn16[:], in1=d16[:], op=ALU.is_lt)
    for b in range(HB):
        nc.vector.tensor_tensor(
            out=GA[:, 2, b, 1:], in0=d16[:, b, :-1], in1=d16[:, b, 1:],
            op=ALU.is_lt)
        nc.vector.tensor_tensor(
            out=GA[:, 3, b, :-1], in0=d16[:, b, 1:], in1=d16[:, b, :-1],
            op=ALU.is_lt)
        nc.vector.tensor_tensor(
            out=GD[:, 0, b, 1:], in0=up16[:, b, :-1], in1=d16[:, b, 1:],
            op=ALU.is_lt)
        nc.vector.tensor_tensor(
            out=GD[:, 1, b, :-1], in0=up16[:, b, 1:], in1=d16[:, b, :-1],
            op=ALU.is_lt)
        nc.vector.tensor_tensor(
            out=GD[:, 2, b, 1:], in0=dn16[:, b, :-1], in1=d16[:, b, 1:],
            op=ALU.is_lt)
        nc.vector.tensor_tensor(
            out=GD[:, 3, b, :-1], in0=dn16[:, b, 1:], in1=d16[:, b, :-1],
            op=ALU.is_lt)
    for b in range(HB):   # x-border clamp: diagonals collapse onto verticals
        nc.scalar.copy(out=GD[:, 0, b, 0:1], in_=GA[:, 0, b, 0:1])
        nc.scalar.copy(out=GD[:, 2, b, 0:1], in_=GA[:, 1, b, 0:1])
        nc.scalar.copy(out=GD[:, 1, b, 255:256], in_=GA[:, 0, b, 255:256])
        nc.scalar.copy(out=GD[:, 3, b, 255:256], in_=GA[:, 1, b, 255:256])

    sa01 = work.tile([128, HB, 256], F16)
    nc.vector.tensor_tensor(out=sa01[:], in0=GA[:, 0], in1=GA[:, 1], op=ALU.add)
    sa23 = work.tile([128, HB, 256], F16)
    nc.vector.tensor_tensor(out=sa23[:], in0=GA[:, 2], in1=GA[:, 3], op=ALU.add)
    SA = work.tile([128, HB, 256], F16)
    nc.vector.tensor_tensor(out=SA[:], in0=sa01[:], in1=sa23[:], op=ALU.add)
    sd01 = work.tile([128, HB, 256], F16)
    nc.vector.tensor_tensor(out=sd01[:], in0=GD[:, 0], in1=GD[:, 1], op=ALU.add)
    sd23 = work.tile([128, HB, 256], F16)
    nc.vector.tensor_tensor(out=sd23[:], in0=GD[:, 2], in1=GD[:, 3], op=ALU.add)
    SD = work.tile([128, HB, 256], F16)
    nc.vector.tensor_tensor(out=SD[:], in0=sd01[:], in1=sd23[:], op=ALU.add)

    sa32 = work.tile([128, HB, 256], F32)
    nc.scalar.activation(sa32[:], SA[:], AF.Copy, bias=0.0, scale=E1)
    s32 = work.tile([128, HB, 256], F32)
    nc.vector.scalar_tensor_tensor(
        out=s32[:], in0=SD[:], scalar=EC, in1=sa32[:],
        op0=ALU.mult, op1=ALU.add)

    if "dbg_s" in dbg:
        for b in range(HB):
            nc.sync.dma_start(out=dbg["dbg_s"][128 * b:128 * (b + 1), :],
                              in_=s32[:, b, :])

    sg = work.tile([128, HB, 256], F32)
    nc.vector.tensor_tensor(out=sg[:], in0=s32[:], in1=img32[:], op=ALU.add)
    lnv = work.tile([128, HB, 256], F32)
    nc.scalar.activation(lnv[:], sg[:], AF.Ln, bias=0.0, scale=LNSCALE)
    outp = work.tile([128, HB, 256], F32)
    nc.vector.scalar_tensor_tensor(
        out=outp[:], in0=lnv[:], scalar=float(-H_PARAM), in1=d16[:],
        op0=ALU.mult, op1=ALU.add)
    nc.vector.tensor_tensor(out=outp[:], in0=outp[:], in1=inv32[:],
                            op=ALU.mult)

    for b in range(HB):
        nc.sync.dma_start(out=out[128 * b:128 * (b + 1), :], in_=outp[:, b, :])

    ctx.close()


_NC_CACHE = None


def _get_nc():
    global _NC_CACHE
    if _NC_CACHE is None:
        _NC_CACHE = _build_program()
    return _NC_CACHE


def kernel(image: np.ndarray) -> np.ndarray:
    """image: (2, 1, 256, 256) float32 -> (2, 1, 256, 256) float32."""
    B, C, Himg, Wimg = image.shape
    flat = np.ascontiguousarray(
        image.reshape(B * C, Himg, Wimg).astype(np.float32))
    n_units = flat.shape[0]
    nc = _get_nc()
    in_maps = [{"img": flat[i % n_units]} for i in range(N_CORES)]
    res = run_bass_kernel_spmd(nc, in_maps, core_ids=list(range(N_CORES)))
    outs = [res.results[i]["out"] for i in range(n_units)]
    return np.stack(outs).reshape(B, C, Himg, Wimg).astype(image.dtype)


if __name__ == "__main__":
    from concourse.bass_interp import CoreSim
    import jax
    cpu = jax.devices("cpu")[0]
    with jax.default_device(cpu):
        import reference as R
        inputs = R.setup_inputs()
        img_np = np.asarray(inputs["image"]).reshape(2, 256, 256)
        expected = np.asarray(R.reference(**inputs)).reshape(2, 256, 256)
    print("reference done", flush=True)
    nc = _get_nc()
    print("program built", flush=True)
    sim = CoreSim(nc)
    sim.tensor("img")[:] = img_np[0]
    sim.simulate()
    got = sim.tensor("out").copy()
    err = np.abs(got - expected[0])
    rel = err.max() / (np.abs(expected[0]).max() + 1e-9)
    print("sim image0: max abs err", err.max(), "rel", rel)


# revision 16
# speedup vs baseline: 1.3935x; 1.0515x over previous
"""Trainium2 Bass kernel for nn_DistanceTransform.

The reference's data-dependent while-loop collapses to a closed form:
    d(p)   = Chebyshev distance from p to the nearest seed
    S(p)   = sum over the 3x3 neighborhood (replicate-clamped) of
             w(dy,dx) * [d(q) < d(p)]
    out(p) = 0 if d(p)==0 else (d(p)-1) - h*ln(S(p))

The Chebyshev DT decomposes exactly into four 1D min-plus passes:
    D* = diagNE(diagSE(seed0))          (cost 1 per step along diagonals)
    d  = min(axisX(D*), axisY(D*))      (cost 1 per step along rows/cols)
Each 1D pass is one forward+backward `tensor_tensor_scan` over all line
blocks concatenated in the free dim, with 256-wide INF separator regions
between blocks (a cross-block leak path costs >= 256 > max(d) = 255, so
leaks never win a min). Diagonal passes run in 45-degree-sheared layouts
produced by DRAM staging buffers with mismatched read/write row pitches;
reads come back through 16-bit DMA-transposes straight into the scan
layout. S(p) uses PE banded matmuls for row-shifted d and DVE is_lt
masks.

Data-parallel over B*C = 2 images: core b computes image b.
"""

import os
import numpy as np

import concourse.bacc as bacc
import concourse.mybir as mybir
from concourse.tile import TileContext
from concourse.masks import make_identity
from concourse.bass_utils import run_bass_kernel_spmd

F32 = mybir.dt.float32
F16 = mybir.dt.float16
I16 = mybir.dt.int16
AF = mybir.ActivationFunctionType
ALU = mybir.AluOpType

H = W = 256
HB = 2
INF = 1536.0
H_PARAM = np.float32(0.35)
E1 = float(np.exp(np.float32(-1.0) / H_PARAM))
EC = float(np.exp(np.float32(-np.sqrt(np.float32(2.0))) / H_PARAM))
LNSCALE = float(np.exp(np.float32(1.0) / H_PARAM))

P1R = 516   # stage1 read pitch (f16); write pitch 515, base 255: c = x+255-y
P2R = 768   # stage2 read pitch (f16); write pitch 770: c' = c+2y-255
P3R = 516   # stage3 read pitch (f16); write pitch 515: x = c'-y

N_CORES = 8


def _build_program():
    nc = bacc.Bacc("TRN2", target_bir_lowering=False, debug=False,
                   num_devices=N_CORES)
    img = nc.dram_tensor("img", [H, W], F32, kind="ExternalInput").ap()
    out = nc.dram_tensor("out", [H, W], F32, kind="ExternalOutput").ap()
    stage1 = nc.dram_tensor("stage1", [256 * P1R + 600], F16).ap()
    stage2 = nc.dram_tensor("stage2", [256 * P2R + 1200], F16).ap()
    stage3 = nc.dram_tensor("stage3", [256 * P3R + 600], F16).ap()

    dbg = {}
    if os.environ.get("DT_DEBUG"):
        for name, shape in [("dbg_d", [H, W]), ("dbg_dstar", [H, W]),
                            ("dbg_s", [H, W])]:
            dbg[name] = nc.dram_tensor(name, shape, F32,
                                       kind="ExternalOutput").ap()

    with TileContext(nc) as tc:
        _emit(nc, tc, img, out, stage1, stage2, stage3, dbg)
    nc.compile()
    return nc


def _emit(nc, tc, img, out, stage1, stage2, stage3, dbg=None):
    import contextlib
    dbg = dbg or {}
    ctx = contextlib.ExitStack()
    const = ctx.enter_context(tc.tile_pool(name="const", bufs=1))
    work = ctx.enter_context(tc.tile_pool(name="work", bufs=1))
    psum = ctx.enter_context(tc.tile_pool(name="psum", bufs=4, space="PSUM"))
    psum2 = ctx.enter_context(tc.tile_pool(name="psum2", bufs=2, space="PSUM"))

    # ---------------- critical-path head: prefills + sheared input --------
    zero16 = const.tile([128, 1032], F16)
    nc.gpsimd.memset(zero16[:], 0.0)
    inf16 = const.tile([128, 1540], F16)
    nc.vector.memset(inf16[:], INF)

    # prefill stage1 with zeros (maps to INF after the seed transform)
    n1 = 256 * P1R
    nc.sync.dma_start(
        out=stage1[:n1].rearrange("(p f) -> p f", p=128),
        in_=zero16[:, : n1 // 128])
    # prefill stage2 with INF
    n2 = 256 * P2R
    nc.scalar.dma_start(
        out=stage2[:n2].rearrange("(p f) -> p f", p=128),
        in_=inf16[:, : n2 // 128])
    # cast band write f32->f16: row y at 255 + 515*y + x; read c = x+255-y
    bandA0 = stage1[255: 255 + 256 * (P1R - 1)].rearrange(
        "(y f) -> y f", f=P1R - 1)[:, :W]
    nc.gpsimd.dma_start(out=bandA0, in_=img[:, :])

    # ---------------- constants / init ------------------------------------
    ident16 = const.tile([128, 128], F16)
    make_identity(nc, ident16[:])

    ones16 = const.tile([128, 256], F16)
    nc.gpsimd.memset(ones16[:], 1.0)

    zero32 = const.tile([128, 512], F32)
    nc.gpsimd.memset(zero32[:], 0.0)

    # sanitize mask (transposed layout): MBT[p, cb, y] = 1 where
    # c' = 128*cb + p is outside [y, y+255]. For cb in {0,1} only c'-y < 0
    # can be invalid; for cb in {2,3} only c'-y > 255.
    VT = const.tile([128, 4, 256], F16)
    nc.vector.memset(VT[:], 1.0)
    for cb in range(4):
        if cb < 2:
            nc.gpsimd.affine_select(   # valid iff (128*cb + p) - y >= 0
                out=VT[:, cb, :], in_=VT[:, cb, :], compare_op=ALU.is_ge,
                fill=0.0, base=128 * cb, pattern=[[-1, 256]],
                channel_multiplier=1)
        else:
            nc.gpsimd.affine_select(   # valid iff 255 - (128*cb + p) + y >= 0
                out=VT[:, cb, :], in_=VT[:, cb, :], compare_op=ALU.is_ge,
                fill=0.0, base=255 - 128 * cb, pattern=[[1, 256]],
                channel_multiplier=-1)
    FILLT = const.tile([128, 4, 256], F16)   # (1 - V) * INF
    nc.scalar.activation(FILLT[:], VT[:], AF.Copy, bias=INF, scale=-INF)

    # banded matrices for row shifts (lhsT: [k, m] = weight of in-row k in
    # out-row m). up: out[m] = in[m-1] (replicate top); down: out[m]=in[m+1].
    def band(tile_ap, diag_base, corner=None):
        nc.gpsimd.memset(tile_ap, 0.0)
        nc.gpsimd.affine_select(
            out=tile_ap, in_=tile_ap, compare_op=ALU.not_equal, fill=1.0,
            base=diag_base, pattern=[[-1, 128]], channel_multiplier=1)
        if corner == "tl":
            nc.gpsimd.affine_select(
                out=tile_ap, in_=tile_ap, compare_op=ALU.not_equal, fill=1.0,
                base=0, pattern=[[1, 128]], channel_multiplier=1)
        elif corner == "br":
            nc.gpsimd.affine_select(
                out=tile_ap, in_=tile_ap, compare_op=ALU.not_equal, fill=1.0,
                base=-254, pattern=[[1, 128]], channel_multiplier=1)

    supA = const.tile([128, 128], F16)
    band(supA[:], 1, corner="tl")
    supB = const.tile([128, 128], F16)
    band(supB[:], 1)
    sdnA = const.tile([128, 128], F16)
    band(sdnA[:], -1)
    sdnB = const.tile([128, 128], F16)
    band(sdnB[:], -1, corner="br")
    e_up = const.tile([128, 128], F16)   # 1 at [k=127, m=0]
    nc.gpsimd.memset(e_up[:], 0.0)
    nc.gpsimd.affine_select(
        out=e_up[:], in_=e_up[:], compare_op=ALU.not_equal, fill=1.0,
        base=127, pattern=[[1, 128]], channel_multiplier=-1)
    e_dn = const.tile([128, 128], F16)   # 1 at [k=0, m=127]
    nc.gpsimd.memset(e_dn[:], 0.0)
    nc.gpsimd.affine_select(
        out=e_dn[:], in_=e_dn[:], compare_op=ALU.not_equal, fill=1.0,
        base=127, pattern=[[-1, 128]], channel_multiplier=1)

    # seed image for the final stage (off critical path)
    img32 = work.tile([128, HB, 256], F32)
    for b in range(HB):
        nc.sync.dma_start(out=img32[:, b, :], in_=img[128 * b:128 * (b + 1), :])
    inv32 = work.tile([128, HB, 256], F32)   # 1 - seed
    nc.scalar.activation(inv32[:], img32[:], AF.Copy, bias=1.0, scale=-1.0)

    # ---------------- shear A + transpose-in ------------------------------
    sk1 = work.tile([128, HB, 512], F16)
    for b in range(HB):
        rd = stage1[128 * b * P1R: (128 * b + 128) * P1R].rearrange(
            "(y f) -> y f", f=P1R)[:, :512]
        eng = nc.sync if b == 0 else nc.scalar
        eng.dma_start(out=sk1[:, b, :], in_=rd)
    sk1t = work.tile([128, HB, 512], F16)
    nc.scalar.activation(sk1t[:], sk1[:], AF.Copy, bias=INF, scale=-INF)

    def transpose_2to4(srct, dstt):
        k = 0
        for cb in range(4):
            for yb in range(2):
                pt = psum.tile([128, 128], F16, tag="tp")
                nc.tensor.transpose(
                    pt[:], srct[:, yb, 128 * cb:128 * (cb + 1)], ident16[:])
                if k % 2 == 0:
                    nc.scalar.copy(out=dstt[:, cb, 128 * yb:128 * (yb + 1)],
                                   in_=pt[:])
                else:
                    nc.vector.tensor_copy(
                        out=dstt[:, cb, 128 * yb:128 * (yb + 1)], in_=pt[:])
                k += 1

    d1 = work.tile([128, 4, 256], F16)
    transpose_2to4(sk1t, d1)

    # ---------------- per-block min-plus pass helper ------------------------
    def minplus(arr, tmp_tag):
        nblk = arr.shape[1]
        for cb in range(nblk):
            tmp = work.tile([128, 256], F16, tag=tmp_tag)
            nc.vector.tensor_tensor_scan(
                out=tmp[:], data0=ones16[:], data1=arr[:, cb, :],
                initial=INF, op0=ALU.add, op1=ALU.min)
            nc.vector.tensor_tensor_scan(
                out=arr[:, cb, ::-1], data0=ones16[:],
                data1=tmp[:, ::-1], initial=INF, op0=ALU.add, op1=ALU.min)

    minplus(d1, "scan1")          # diag SE pass (lines c = x-y+255)

    # ---------------- transpose-back + shear B -----------------------------
    def transpose_4to2(src, dst):
        k = 0
        for yb in range(2):
            for cb in range(4):
                pt = psum.tile([128, 128], F16, tag="tp")
                nc.tensor.transpose(
                    pt[:], src[:, cb, 128 * yb:128 * (yb + 1)], ident16[:])
                if k % 2 == 0:
                    nc.scalar.copy(out=dst[:, yb, 128 * cb:128 * (cb + 1)],
                                   in_=pt[:])
                else:
                    nc.vector.tensor_copy(
                        out=dst[:, yb, 128 * cb:128 * (cb + 1)], in_=pt[:])
                k += 1

    sk1b = work.tile([128, HB, 512], F16)
    transpose_4to2(d1, sk1b)

    # write(y, c) at 770*y + c ; read(y, c') at 768*y + 255 + c'
    for b in range(HB):
        wr = stage2[128 * b * (P2R + 2): (128 * b + 128) * (P2R + 2)].rearrange(
            "(y f) -> y f", f=P2R + 2)[:, :512]
        nc.sync.dma_start(out=wr, in_=sk1b[:, b, :])
    sk2 = work.tile([128, HB, 512], F16)
    for b in range(HB):
        rd = stage2[255 + 128 * b * P2R: 255 + (128 * b + 128) * P2R].rearrange(
            "(y f) -> y f", f=P2R)[:, :512]
        eng = nc.sync if b == 0 else nc.scalar
        eng.dma_start(out=sk2[:, b, :], in_=rd)
    d2 = work.tile([128, 4, 256], F16)
    transpose_2to4(sk2, d2)
    nc.vector.tensor_tensor(out=d2[:], in0=d2[:], in1=VT[:], op=ALU.mult)
    nc.vector.tensor_tensor(out=d2[:], in0=d2[:], in1=FILLT[:], op=ALU.add)

    minplus(d2, "scan2")          # diag NE pass (lines c' = x+y)

    sk2b = work.tile([128, HB, 512], F16)
    transpose_4to2(d2, sk2b)

    # ---------------- unshear C -------------------------------------------
    # write(y, c') at 515*y + c' ; read(y, x) at 516*y + x  (x = c'-y)
    for b in range(HB):
        wr = stage3[128 * b * (P3R - 1): (128 * b + 128) * (P3R - 1)].rearrange(
            "(y f) -> y f", f=P3R - 1)[:, :512]
        nc.sync.dma_start(out=wr, in_=sk2b[:, b, :])
    dstar = work.tile([128, HB, 256], F16)
    for b in range(HB):
        rd = stage3[128 * b * P3R: (128 * b + 128) * P3R].rearrange(
            "(y f) -> y f", f=P3R)[:, :256]
        nc.sync.dma_start(out=dstar[:, b, :], in_=rd)
    dstT = work.tile([128, HB, 256], F16)
    k = 0
    for xb in range(2):
        for yb in range(2):
            pt = psum.tile([128, 128], F16, tag="tp")
            nc.tensor.transpose(
                pt[:], dstar[:, yb, 128 * xb:128 * (xb + 1)], ident16[:])
            if k % 2 == 0:
                nc.scalar.copy(out=dstT[:, xb, 128 * yb:128 * (yb + 1)],
                               in_=pt[:])
            else:
                nc.vector.tensor_copy(
                    out=dstT[:, xb, 128 * yb:128 * (yb + 1)], in_=pt[:])
            k += 1

    if "dbg_dstar" in dbg:
        t = work.tile([128, HB, 256], F32, tag="dbg1")
        nc.vector.tensor_copy(out=t[:], in_=dstar[:])
        for b in range(HB):
            nc.sync.dma_start(out=dbg["dbg_dstar"][128 * b:128 * (b + 1), :],
                              in_=t[:, b, :])

    # ---------------- axis passes ------------------------------------------
    minplus(dstar, "scan3")       # axisX in image layout
    minplus(dstT, "scan4")        # axisY in transposed layout
    dy = work.tile([128, HB, 256], F16)
    k = 0
    for yb in range(2):
        for xb in range(2):
            pt = psum.tile([128, 128], F16, tag="tp")
            nc.tensor.transpose(
                pt[:], dstT[:, xb, 128 * yb:128 * (yb + 1)], ident16[:])
            if k % 2 == 0:
                nc.scalar.copy(out=dy[:, yb, 128 * xb:128 * (xb + 1)],
                               in_=pt[:])
            else:
                nc.vector.tensor_copy(out=dy[:, yb, 128 * xb:128 * (xb + 1)],
                                      in_=pt[:])
            k += 1

    d16 = work.tile([128, HB, 256], F16)
    nc.vector.tensor_tensor(out=d16[:], in0=dstar[:], in1=dy[:], op=ALU.min)

    if "dbg_d" in dbg:
        t = work.tile([128, HB, 256], F32, tag="dbg2")
        nc.vector.tensor_copy(out=t[:], in_=d16[:])
        for b in range(HB):
            nc.sync.dma_start(out=dbg["dbg_d"][128 * b:128 * (b + 1), :],
                              in_=t[:, b, :])

    # ---------------- S stage ---------------------------------------------
    pup = psum2.tile([128, HB, 256], F32, tag="pup")
    nc.tensor.matmul(pup[:, 0, :], supA[:], d16[:, 0, :], start=True, stop=True)
    nc.tensor.matmul(pup[:, 1, :], supB[:], d16[:, 1, :], start=True, stop=False)
    nc.tensor.matmul(pup[:, 1, :], e_up[:], d16[:, 0, :], start=False, stop=True)
    pdn = psum2.tile([128, HB, 256], F32, tag="pdn")
    nc.tensor.matmul(pdn[:, 0, :], sdnA[:], d16[:, 0, :], start=True, stop=False)
    nc.tensor.matmul(pdn[:, 0, :], e_dn[:], d16[:, 1, :], start=False, stop=True)
    nc.tensor.matmul(pdn[:, 1, :], sdnB[:], d16[:, 1, :], start=True, stop=True)
    up16 = work.tile([128, HB, 256], F16)
    nc.scalar.copy(out=up16[:], in_=pup[:])
    dn16 = work.tile([128, HB, 256], F16)
    nc.scalar.copy(out=dn16[:], in_=pdn[:])

    GA = work.tile([128, 4, HB, 256], F16)
    GD = work.tile([128, 4, HB, 256], F16)
    for b in range(HB):   # zero only the never-written border columns
        nc.gpsimd.memset(GA[:, 2, b, 0:1], 0.0)
        nc.gpsimd.memset(GA[:, 3, b, 255:256], 0.0)

    nc.vector.tensor_tensor(out=GA[:, 0], in0=up16[:], in1=d16[:], op=ALU.is_lt)
    nc.vector.tensor_tensor(out=GA[:, 1], in0=dn16[:], in1=d16[:], op=ALU.is_lt)
    for b in range(HB):
        nc.vector.tensor_tensor(
            out=GA[:, 2, b, 1:], in0=d16[:, b, :-1], in1=d16[:, b, 1:],
            op=ALU.is_lt)
        nc.vector.tensor_tensor(
            out=GA[:, 3, b, :-1], in0=d16[:, b, 1:], in1=d16[:, b, :-1],
            op=ALU.is_lt)
        nc.vector.tensor_tensor(
            out=GD[:, 0, b, 1:], in0=up16[:, b, :-1], in1=d16[:, b, 1:],
            op=ALU.is_lt)
        nc.vector.tensor_tensor(
            out=GD[:, 1, b, :-1], in0=up16[:, b, 1:], in1=d16[:, b, :-1],
            op=ALU.is_lt)
        nc.vector.tensor_tensor(
            out=GD[:, 2, b, 1:], in0=dn16[:, b, :-1], in1=d16[:, b, 1:],
            op=ALU.is_lt)
        nc.vector.tensor_tensor(
            out=GD[:, 3, b, :-1], in0=dn16[:, b, 1:], in1=d16[:, b, :-1],
            op=ALU.is_lt)
    for b in range(HB):   # x-border clamp: diagonals collapse onto verticals
        nc.scalar.copy(out=GD[:, 0, b, 0:1], in_=GA[:, 0, b, 0:1])
        nc.scalar.copy(out=GD[:, 2, b, 0:1], in_=GA[:, 1, b, 0:1])
        nc.scalar.copy(out=GD[:, 1, b, 255:256], in_=GA[:, 0, b, 255:256])
        nc.scalar.copy(out=GD[:, 3, b, 255:256], in_=GA[:, 1, b, 255:256])

    sa01 = work.tile([128, HB, 256], F16)
    nc.vector.tensor_tensor(out=sa01[:], in0=GA[:, 0], in1=GA[:, 1], op=ALU.add)
    sa23 = work.tile([128, HB, 256], F16)
    nc.vector.tensor_tensor(out=sa23[:], in0=GA[:, 2], in1=GA[:, 3], op=ALU.add)
    SA = work.tile([128, HB, 256], F16)
    nc.vector.tensor_tensor(out=SA[:], in0=sa01[:], in1=sa23[:], op=ALU.add)
    sd01 = work.tile([128, HB, 256], F16)
    nc.vector.tensor_tensor(out=sd01[:], in0=GD[:, 0], in1=GD[:, 1], op=ALU.add)
    sd23 = work.tile([128, HB, 256], F16)
    nc.vector.tensor_tensor(out=sd23[:], in0=GD[:, 2], in1=GD[:, 3], op=ALU.add)
    SD = work.tile([128, HB, 256], F16)
    nc.vector.tensor_tensor(out=SD[:], in0=sd01[:], in1=sd23[:], op=ALU.add)

    sa32 = work.tile([128, HB, 256], F32)
    nc.scalar.activation(sa32[:], SA[:], AF.Copy, bias=0.0, scale=E1)
    s32 = work.tile([128, HB, 256], F32)
    nc.vector.scalar_tensor_tensor(
        out=s32[:], in0=SD[:], scalar=EC, in1=sa32[:],
        op0=ALU.mult, op1=ALU.add)

    if "dbg_s" in dbg:
        for b in range(HB):
            nc.sync.dma_start(out=dbg["dbg_s"][128 * b:128 * (b + 1), :],
                              in_=s32[:, b, :])

    sg = work.tile([128, HB, 256], F32)
    nc.vector.tensor_tensor(out=sg[:], in0=s32[:], in1=img32[:], op=ALU.add)
    lnv = work.tile([128, HB, 256], F32)
    nc.scalar.activation(lnv[:], sg[:], AF.Ln, bias=0.0, scale=LNSCALE)
    outp = work.tile([128, HB, 256], F32)
    nc.vector.scalar_tensor_tensor(
        out=outp[:], in0=lnv[:], scalar=float(-H_PARAM), in1=d16[:],
        op0=ALU.mult, op1=ALU.add)
    nc.vector.tensor_tensor(out=outp[:], in0=outp[:], in1=inv32[:],
                            op=ALU.mult)

    for b in range(HB):
        nc.sync.dma_start(out=out[128 * b:128 * (b + 1), :], in_=outp[:, b, :])

    ctx.close()


_NC_CACHE = None


def _get_nc():
    global _NC_CACHE
    if _NC_CACHE is None:
        _NC_CACHE = _build_program()
    return _NC_CACHE


def kernel(image: np.ndarray) -> np.ndarray:
    """image: (2, 1, 256, 256) float32 -> (2, 1, 256, 256) float32."""
    B, C, Himg, Wimg = image.shape
    flat = np.ascontiguousarray(
        image.reshape(B * C, Himg, Wimg).astype(np.float32))
    n_units = flat.shape[0]
    nc = _get_nc()
    in_maps = [{"img": flat[i % n_units]} for i in range(N_CORES)]
    res = run_bass_kernel_spmd(nc, in_maps, core_ids=list(range(N_CORES)))
    outs = [res.results[i]["out"] for i in range(n_units)]
    return np.stack(outs).reshape(B, C, Himg, Wimg).astype(image.dtype)


if __name__ == "__main__":
    from concourse.bass_interp import CoreSim
    import jax
    cpu = jax.devices("cpu")[0]
    with jax.default_device(cpu):
        import reference as R
        inputs = R.setup_inputs()
        img_np = np.asarray(inputs["image"]).reshape(2, 256, 256)
        expected = np.asarray(R.reference(**inputs)).reshape(2, 256, 256)
    print("reference done", flush=True)
    nc = _get_nc()
    print("program built", flush=True)
    sim = CoreSim(nc)
    sim.tensor("img")[:] = img_np[0]
    sim.simulate()
    got = sim.tensor("out").copy()
    err = np.abs(got - expected[0])
    rel = err.max() / (np.abs(expected[0]).max() + 1e-9)
    print("sim image0: max abs err", err.max(), "rel", rel)


# revision 17
# speedup vs baseline: 1.4041x; 1.0077x over previous
"""Trainium2 Bass kernel for nn_DistanceTransform.

The reference's data-dependent while-loop collapses to a closed form:
    d(p)   = Chebyshev distance from p to the nearest seed
    S(p)   = sum over the 3x3 neighborhood (replicate-clamped) of
             w(dy,dx) * [d(q) < d(p)]
    out(p) = 0 if d(p)==0 else (d(p)-1) - h*ln(S(p))

The Chebyshev DT decomposes exactly into four 1D min-plus passes:
    D* = diagNE(diagSE(seed0))          (cost 1 per step along diagonals)
    d  = min(axisX(D*), axisY(D*))      (cost 1 per step along rows/cols)
Each 1D pass is one forward+backward `tensor_tensor_scan` over all line
blocks concatenated in the free dim, with 256-wide INF separator regions
between blocks (a cross-block leak path costs >= 256 > max(d) = 255, so
leaks never win a min). Diagonal passes run in 45-degree-sheared layouts
produced by DRAM staging buffers with mismatched read/write row pitches;
reads come back through 16-bit DMA-transposes straight into the scan
layout. S(p) uses PE banded matmuls for row-shifted d and DVE is_lt
masks.

Data-parallel over B*C = 2 images: core b computes image b.
"""

import os
import numpy as np

import concourse.bacc as bacc
import concourse.mybir as mybir
from concourse.tile import TileContext
from concourse.masks import make_identity
from concourse.bass_utils import run_bass_kernel_spmd

F32 = mybir.dt.float32
F16 = mybir.dt.float16
I16 = mybir.dt.int16
AF = mybir.ActivationFunctionType
ALU = mybir.AluOpType

H = W = 256
HB = 2
INF = 1536.0
H_PARAM = np.float32(0.35)
E1 = float(np.exp(np.float32(-1.0) / H_PARAM))
EC = float(np.exp(np.float32(-np.sqrt(np.float32(2.0))) / H_PARAM))
LNSCALE = float(np.exp(np.float32(1.0) / H_PARAM))

P1R = 516   # stage1 read pitch (f16); write pitch 515, base 255: c = x+255-y
P2R = 768   # stage2 read pitch (f16); write pitch 770: c' = c+2y-255
P3R = 516   # stage3 read pitch (f16); write pitch 515: x = c'-y

N_CORES = 8


def _build_program():
    nc = bacc.Bacc("TRN2", target_bir_lowering=False, debug=False,
                   num_devices=N_CORES)
    img = nc.dram_tensor("img", [H, W], F32, kind="ExternalInput").ap()
    out = nc.dram_tensor("out", [H, W], F32, kind="ExternalOutput").ap()
    stage1 = nc.dram_tensor("stage1", [256 * P1R + 600], F16).ap()
    stage2 = nc.dram_tensor("stage2", [256 * P2R + 1200], F16).ap()
    stage3 = nc.dram_tensor("stage3", [256 * P3R + 600], F16).ap()

    dbg = {}
    if os.environ.get("DT_DEBUG"):
        for name, shape in [("dbg_d", [H, W]), ("dbg_dstar", [H, W]),
                            ("dbg_s", [H, W])]:
            dbg[name] = nc.dram_tensor(name, shape, F32,
                                       kind="ExternalOutput").ap()

    with TileContext(nc) as tc:
        _emit(nc, tc, img, out, stage1, stage2, stage3, dbg)
    nc.compile()
    return nc


def _emit(nc, tc, img, out, stage1, stage2, stage3, dbg=None):
    import contextlib
    dbg = dbg or {}
    ctx = contextlib.ExitStack()
    const = ctx.enter_context(tc.tile_pool(name="const", bufs=1))
    work = ctx.enter_context(tc.tile_pool(name="work", bufs=1))
    psum = ctx.enter_context(tc.tile_pool(name="psum", bufs=4, space="PSUM"))
    psum2 = ctx.enter_context(tc.tile_pool(name="psum2", bufs=2, space="PSUM"))

    # ---------------- critical-path head ----------------------------------
    inf16 = const.tile([128, 1540], F16)
    nc.vector.memset(inf16[:], INF)

    # load the seed image (also used by the final stage)
    img32 = work.tile([128, HB, 256], F32)
    nc.sync.dma_start(out=img32[:, 0, :], in_=img[0:128, :])
    nc.scalar.dma_start(out=img32[:, 1, :], in_=img[128:256, :])

    # prefill stage1 with INF (field is post-transform: seeds 0, empty INF)
    n1 = 256 * P1R
    nc.sync.dma_start(
        out=stage1[:n1].rearrange("(p f) -> p f", p=128),
        in_=inf16[:, : n1 // 128])
    # prefill stage2 with INF
    n2 = 256 * P2R
    nc.scalar.dma_start(
        out=stage2[:n2].rearrange("(p f) -> p f", p=128),
        in_=inf16[:, : n2 // 128])

    # seed transform on-chip: 0 -> INF, 1 -> 0 (fp16)
    d0 = work.tile([128, HB, 256], F16)
    nc.scalar.activation(d0[:], img32[:], AF.Copy, bias=INF, scale=-INF)
    # sheared band write: row y at 255 + 515*y + x; read c = x+255-y
    bandA = stage1[255: 255 + 256 * (P1R - 1)].rearrange(
        "(y f) -> y f", f=P1R - 1)[:, :W]
    nc.sync.dma_start(out=bandA[0:128, :], in_=d0[:, 0, :])
    nc.scalar.dma_start(out=bandA[128:256, :], in_=d0[:, 1, :])

    # ---------------- constants / init ------------------------------------
    ident16 = const.tile([128, 128], F16)
    make_identity(nc, ident16[:])

    ones16 = const.tile([128, 256], F16)
    nc.gpsimd.memset(ones16[:], 1.0)

    zero32 = const.tile([128, 512], F32)
    nc.gpsimd.memset(zero32[:], 0.0)

    # sanitize mask (transposed layout): MBT[p, cb, y] = 1 where
    # c' = 128*cb + p is outside [y, y+255]. For cb in {0,1} only c'-y < 0
    # can be invalid; for cb in {2,3} only c'-y > 255.
    VT = const.tile([128, 4, 256], F16)
    nc.vector.memset(VT[:], 1.0)
    for cb in range(4):
        if cb < 2:
            nc.gpsimd.affine_select(   # valid iff (128*cb + p) - y >= 0
                out=VT[:, cb, :], in_=VT[:, cb, :], compare_op=ALU.is_ge,
                fill=0.0, base=128 * cb, pattern=[[-1, 256]],
                channel_multiplier=1)
        else:
            nc.gpsimd.affine_select(   # valid iff 255 - (128*cb + p) + y >= 0
                out=VT[:, cb, :], in_=VT[:, cb, :], compare_op=ALU.is_ge,
                fill=0.0, base=255 - 128 * cb, pattern=[[1, 256]],
                channel_multiplier=-1)
    FILLT = const.tile([128, 4, 256], F16)   # (1 - V) * INF
    nc.scalar.activation(FILLT[:], VT[:], AF.Copy, bias=INF, scale=-INF)

    # banded matrices for row shifts (lhsT: [k, m] = weight of in-row k in
    # out-row m). up: out[m] = in[m-1] (replicate top); down: out[m]=in[m+1].
    def band(tile_ap, diag_base, corner=None):
        nc.gpsimd.memset(tile_ap, 0.0)
        nc.gpsimd.affine_select(
            out=tile_ap, in_=tile_ap, compare_op=ALU.not_equal, fill=1.0,
            base=diag_base, pattern=[[-1, 128]], channel_multiplier=1)
        if corner == "tl":
            nc.gpsimd.affine_select(
                out=tile_ap, in_=tile_ap, compare_op=ALU.not_equal, fill=1.0,
                base=0, pattern=[[1, 128]], channel_multiplier=1)
        elif corner == "br":
            nc.gpsimd.affine_select(
                out=tile_ap, in_=tile_ap, compare_op=ALU.not_equal, fill=1.0,
                base=-254, pattern=[[1, 128]], channel_multiplier=1)

    supA = const.tile([128, 128], F16)
    band(supA[:], 1, corner="tl")
    supB = const.tile([128, 128], F16)
    band(supB[:], 1)
    sdnA = const.tile([128, 128], F16)
    band(sdnA[:], -1)
    sdnB = const.tile([128, 128], F16)
    band(sdnB[:], -1, corner="br")
    e_up = const.tile([128, 128], F16)   # 1 at [k=127, m=0]
    nc.gpsimd.memset(e_up[:], 0.0)
    nc.gpsimd.affine_select(
        out=e_up[:], in_=e_up[:], compare_op=ALU.not_equal, fill=1.0,
        base=127, pattern=[[1, 128]], channel_multiplier=-1)
    e_dn = const.tile([128, 128], F16)   # 1 at [k=0, m=127]
    nc.gpsimd.memset(e_dn[:], 0.0)
    nc.gpsimd.affine_select(
        out=e_dn[:], in_=e_dn[:], compare_op=ALU.not_equal, fill=1.0,
        base=127, pattern=[[-1, 128]], channel_multiplier=1)

    inv32 = work.tile([128, HB, 256], F32)   # 1 - seed
    nc.scalar.activation(inv32[:], img32[:], AF.Copy, bias=1.0, scale=-1.0)

    # ---------------- shear A + transpose-in ------------------------------
    sk1t = work.tile([128, HB, 512], F16)
    for b in range(HB):
        rd = stage1[128 * b * P1R: (128 * b + 128) * P1R].rearrange(
            "(y f) -> y f", f=P1R)[:, :512]
        eng = nc.sync if b == 0 else nc.scalar
        eng.dma_start(out=sk1t[:, b, :], in_=rd)

    def transpose_2to4(srct, dstt):
        k = 0
        for cb in range(4):
            for yb in range(2):
                pt = psum.tile([128, 128], F16, tag="tp")
                nc.tensor.transpose(
                    pt[:], srct[:, yb, 128 * cb:128 * (cb + 1)], ident16[:])
                if k % 2 == 0:
                    nc.scalar.copy(out=dstt[:, cb, 128 * yb:128 * (yb + 1)],
                                   in_=pt[:])
                else:
                    nc.vector.tensor_copy(
                        out=dstt[:, cb, 128 * yb:128 * (yb + 1)], in_=pt[:])
                k += 1

    d1 = work.tile([128, 4, 256], F16)
    transpose_2to4(sk1t, d1)

    # ---------------- per-block min-plus pass helper ------------------------
    def minplus(arr, tmp_tag):
        nblk = arr.shape[1]
        for cb in range(nblk):
            tmp = work.tile([128, 256], F16, tag=tmp_tag)
            nc.vector.tensor_tensor_scan(
                out=tmp[:], data0=ones16[:], data1=arr[:, cb, :],
                initial=INF, op0=ALU.add, op1=ALU.min)
            nc.vector.tensor_tensor_scan(
                out=arr[:, cb, ::-1], data0=ones16[:],
                data1=tmp[:, ::-1], initial=INF, op0=ALU.add, op1=ALU.min)

    minplus(d1, "scan1")          # diag SE pass (lines c = x-y+255)

    # ---------------- transpose-back + shear B -----------------------------
    def transpose_4to2(src, dst):
        k = 0
        for yb in range(2):
            for cb in range(4):
                pt = psum.tile([128, 128], F16, tag="tp")
                nc.tensor.transpose(
                    pt[:], src[:, cb, 128 * yb:128 * (yb + 1)], ident16[:])
                if k % 2 == 0:
                    nc.scalar.copy(out=dst[:, yb, 128 * cb:128 * (cb + 1)],
                                   in_=pt[:])
                else:
                    nc.vector.tensor_copy(
                        out=dst[:, yb, 128 * cb:128 * (cb + 1)], in_=pt[:])
                k += 1

    sk1b = work.tile([128, HB, 512], F16)
    transpose_4to2(d1, sk1b)

    # write(y, c) at 770*y + c ; read(y, c') at 768*y + 255 + c'
    for b in range(HB):
        wr = stage2[128 * b * (P2R + 2): (128 * b + 128) * (P2R + 2)].rearrange(
            "(y f) -> y f", f=P2R + 2)[:, :512]
        nc.sync.dma_start(out=wr, in_=sk1b[:, b, :])
    sk2 = work.tile([128, HB, 512], F16)
    for b in range(HB):
        rd = stage2[255 + 128 * b * P2R: 255 + (128 * b + 128) * P2R].rearrange(
            "(y f) -> y f", f=P2R)[:, :512]
        eng = nc.sync if b == 0 else nc.scalar
        eng.dma_start(out=sk2[:, b, :], in_=rd)
    d2 = work.tile([128, 4, 256], F16)
    transpose_2to4(sk2, d2)
    nc.vector.tensor_tensor(out=d2[:], in0=d2[:], in1=VT[:], op=ALU.mult)
    nc.vector.tensor_tensor(out=d2[:], in0=d2[:], in1=FILLT[:], op=ALU.add)

    minplus(d2, "scan2")          # diag NE pass (lines c' = x+y)

    sk2b = work.tile([128, HB, 512], F16)
    transpose_4to2(d2, sk2b)

    # ---------------- unshear C -------------------------------------------
    # write(y, c') at 515*y + c' ; read(y, x) at 516*y + x  (x = c'-y)
    for b in range(HB):
        wr = stage3[128 * b * (P3R - 1): (128 * b + 128) * (P3R - 1)].rearrange(
            "(y f) -> y f", f=P3R - 1)[:, :512]
        nc.sync.dma_start(out=wr, in_=sk2b[:, b, :])
    dstar = work.tile([128, HB, 256], F16)
    for b in range(HB):
        rd = stage3[128 * b * P3R: (128 * b + 128) * P3R].rearrange(
            "(y f) -> y f", f=P3R)[:, :256]
        nc.sync.dma_start(out=dstar[:, b, :], in_=rd)
    dstT = work.tile([128, HB, 256], F16)
    k = 0
    for xb in range(2):
        for yb in range(2):
            pt = psum.tile([128, 128], F16, tag="tp")
            nc.tensor.transpose(
                pt[:], dstar[:, yb, 128 * xb:128 * (xb + 1)], ident16[:])
            if k % 2 == 0:
                nc.scalar.copy(out=dstT[:, xb, 128 * yb:128 * (yb + 1)],
                               in_=pt[:])
            else:
                nc.vector.tensor_copy(
                    out=dstT[:, xb, 128 * yb:128 * (yb + 1)], in_=pt[:])
            k += 1

    if "dbg_dstar" in dbg:
        t = work.tile([128, HB, 256], F32, tag="dbg1")
        nc.vector.tensor_copy(out=t[:], in_=dstar[:])
        for b in range(HB):
            nc.sync.dma_start(out=dbg["dbg_dstar"][128 * b:128 * (b + 1), :],
                              in_=t[:, b, :])

    # ---------------- axis passes ------------------------------------------
    minplus(dstar, "scan3")       # axisX in image layout
    minplus(dstT, "scan4")        # axisY in transposed layout
    dy = work.tile([128, HB, 256], F16)
    k = 0
    for yb in range(2):
        for xb in range(2):
            pt = psum.tile([128, 128], F16, tag="tp")
            nc.tensor.transpose(
                pt[:], dstT[:, xb, 128 * yb:128 * (yb + 1)], ident16[:])
            if k % 2 == 0:
                nc.scalar.copy(out=dy[:, yb, 128 * xb:128 * (xb + 1)],
                               in_=pt[:])
            else:
                nc.vector.tensor_copy(out=dy[:, yb, 128 * xb:128 * (xb + 1)],
                                      in_=pt[:])
            k += 1

    d16 = work.tile([128, HB, 256], F16)
    nc.vector.tensor_tensor(out=d16[:], in0=dstar[:], in1=dy[:], op=ALU.min)

    if "dbg_d" in dbg:
        t = work.tile([128, HB, 256], F32, tag="dbg2")
        nc.vector.tensor_copy(out=t[:], in_=d16[:])
        for b in range(HB):
            nc.sync.dma_start(out=dbg["dbg_d"][128 * b:128 * (b + 1), :],
                              in_=t[:, b, :])

    # ---------------- S stage ---------------------------------------------
    pup = psum2.tile([128, HB, 256], F32, tag="pup")
    nc.tensor.matmul(pup[:, 0, :], supA[:], d16[:, 0, :], start=True, stop=True)
    nc.tensor.matmul(pup[:, 1, :], supB[:], d16[:, 1, :], start=True, stop=False)
    nc.tensor.matmul(pup[:, 1, :], e_up[:], d16[:, 0, :], start=False, stop=True)
    pdn = psum2.tile([128, HB, 256], F32, tag="pdn")
    nc.tensor.matmul(pdn[:, 0, :], sdnA[:], d16[:, 0, :], start=True, stop=False)
    nc.tensor.matmul(pdn[:, 0, :], e_dn[:], d16[:, 1, :], start=False, stop=True)
    nc.tensor.matmul(pdn[:, 1, :], sdnB[:], d16[:, 1, :], start=True, stop=True)
    up16 = work.tile([128, HB, 256], F16)
    nc.scalar.copy(out=up16[:], in_=pup[:])
    dn16 = work.tile([128, HB, 256], F16)
    nc.scalar.copy(out=dn16[:], in_=pdn[:])

    GA = work.tile([128, 4, HB, 256], F16)
    GD = work.tile([128, 4, HB, 256], F16)
    for b in range(HB):   # zero only the never-written border columns
        nc.gpsimd.memset(GA[:, 2, b, 0:1], 0.0)
        nc.gpsimd.memset(GA[:, 3, b, 255:256], 0.0)

    nc.vector.tensor_tensor(out=GA[:, 0], in0=up16[:], in1=d16[:], op=ALU.is_lt)
    nc.vector.tensor_tensor(out=GA[:, 1], in0=dn16[:], in1=d16[:], op=ALU.is_lt)
    for b in range(HB):
        nc.vector.tensor_tensor(
            out=GA[:, 2, b, 1:], in0=d16[:, b, :-1], in1=d16[:, b, 1:],
            op=ALU.is_lt)
        nc.vector.tensor_tensor(
            out=GA[:, 3, b, :-1], in0=d16[:, b, 1:], in1=d16[:, b, :-1],
            op=ALU.is_lt)
        nc.vector.tensor_tensor(
            out=GD[:, 0, b, 1:], in0=up16[:, b, :-1], in1=d16[:, b, 1:],
            op=ALU.is_lt)
        nc.vector.tensor_tensor(
            out=GD[:, 1, b, :-1], in0=up16[:, b, 1:], in1=d16[:, b, :-1],
            op=ALU.is_lt)
        nc.vector.tensor_tensor(
            out=GD[:, 2, b, 1:], in0=dn16[:, b, :-1], in1=d16[:, b, 1:],
            op=ALU.is_lt)
        nc.vector.tensor_tensor(
            out=GD[:, 3, b, :-1], in0=dn16[:, b, 1:], in1=d16[:, b, :-1],
            op=ALU.is_lt)
    for b in range(HB):   # x-border clamp: diagonals collapse onto verticals
        nc.scalar.copy(out=GD[:, 0, b, 0:1], in_=GA[:, 0, b, 0:1])
        nc.scalar.copy(out=GD[:, 2, b, 0:1], in_=GA[:, 1, b, 0:1])
        nc.scalar.copy(out=GD[:, 1, b, 255:256], in_=GA[:, 0, b, 255:256])
        nc.scalar.copy(out=GD[:, 3, b, 255:256], in_=GA[:, 1, b, 255:256])

    sa01 = work.tile([128, HB, 256], F16)
    nc.vector.tensor_tensor(out=sa01[:], in0=GA[:, 0], in1=GA[:, 1], op=ALU.add)
    sa23 = work.tile([128, HB, 256], F16)
    nc.vector.tensor_tensor(out=sa23[:], in0=GA[:, 2], in1=GA[:, 3], op=ALU.add)
    SA = work.tile([128, HB, 256], F16)
    nc.vector.tensor_tensor(out=SA[:], in0=sa01[:], in1=sa23[:], op=ALU.add)
    sd01 = work.tile([128, HB, 256], F16)
    nc.vector.tensor_tensor(out=sd01[:], in0=GD[:, 0], in1=GD[:, 1], op=ALU.add)
    sd23 = work.tile([128, HB, 256], F16)
    nc.vector.tensor_tensor(out=sd23[:], in0=GD[:, 2], in1=GD[:, 3], op=ALU.add)
    SD = work.tile([128, HB, 256], F16)
    nc.vector.tensor_tensor(out=SD[:], in0=sd01[:], in1=sd23[:], op=ALU.add)

    sa32 = work.tile([128, HB, 256], F32)
    nc.scalar.activation(sa32[:], SA[:], AF.Copy, bias=0.0, scale=E1)
    s32 = work.tile([128, HB, 256], F32)
    nc.vector.scalar_tensor_tensor(
        out=s32[:], in0=SD[:], scalar=EC, in1=sa32[:],
        op0=ALU.mult, op1=ALU.add)

    if "dbg_s" in dbg:
        for b in range(HB):
            nc.sync.dma_start(out=dbg["dbg_s"][128 * b:128 * (b + 1), :],
                              in_=s32[:, b, :])

    sg = work.tile([128, HB, 256], F32)
    nc.vector.tensor_tensor(out=sg[:], in0=s32[:], in1=img32[:], op=ALU.add)
    lnv = work.tile([128, HB, 256], F32)
    nc.scalar.activation(lnv[:], sg[:], AF.Ln, bias=0.0, scale=LNSCALE)
    outp = work.tile([128, HB, 256], F32)
    nc.vector.scalar_tensor_tensor(
        out=outp[:], in0=lnv[:], scalar=float(-H_PARAM), in1=d16[:],
        op0=ALU.mult, op1=ALU.add)
    nc.vector.tensor_tensor(out=outp[:], in0=outp[:], in1=inv32[:],
                            op=ALU.mult)

    for b in range(HB):
        nc.sync.dma_start(out=out[128 * b:128 * (b + 1), :], in_=outp[:, b, :])

    ctx.close()


_NC_CACHE = None


def _get_nc():
    global _NC_CACHE
    if _NC_CACHE is None:
        _NC_CACHE = _build_program()
    return _NC_CACHE


def kernel(image: np.ndarray) -> np.ndarray:
    """image: (2, 1, 256, 256) float32 -> (2, 1, 256, 256) float32."""
    B, C, Himg, Wimg = image.shape
    flat = np.ascontiguousarray(
        image.reshape(B * C, Himg, Wimg).astype(np.float32))
    n_units = flat.shape[0]
    nc = _get_nc()
    in_maps = [{"img": flat[i % n_units]} for i in range(N_CORES)]
    res = run_bass_kernel_spmd(nc, in_maps, core_ids=list(range(N_CORES)))
    outs = [res.results[i]["out"] for i in range(n_units)]
    return np.stack(outs).reshape(B, C, Himg, Wimg).astype(image.dtype)


if __name__ == "__main__":
    from concourse.bass_interp import CoreSim
    import jax
    cpu = jax.devices("cpu")[0]
    with jax.default_device(cpu):
        import reference as R
        inputs = R.setup_inputs()
        img_np = np.asarray(inputs["image"]).reshape(2, 256, 256)
        expected = np.asarray(R.reference(**inputs)).reshape(2, 256, 256)
    print("reference done", flush=True)
    nc = _get_nc()
    print("program built", flush=True)
    sim = CoreSim(nc)
    sim.tensor("img")[:] = img_np[0]
    sim.simulate()
    got = sim.tensor("out").copy()
    err = np.abs(got - expected[0])
    rel = err.max() / (np.abs(expected[0]).max() + 1e-9)
    print("sim image0: max abs err", err.max(), "rel", rel)


# revision 19
# speedup vs baseline: 1.4285x; 1.0173x over previous
"""Trainium2 Bass kernel for nn_DistanceTransform.

The reference's data-dependent while-loop collapses to a closed form:
    d(p)   = Chebyshev distance from p to the nearest seed
    S(p)   = sum over the 3x3 neighborhood (replicate-clamped) of
             w(dy,dx) * [d(q) < d(p)]
    out(p) = 0 if d(p)==0 else (d(p)-1) - h*ln(S(p))

The Chebyshev DT decomposes exactly into four 1D min-plus passes:
    D* = diagNE(diagSE(seed0))          (cost 1 per step along diagonals)
    d  = min(axisX(D*), axisY(D*))      (cost 1 per step along rows/cols)
Each 1D pass is one forward+backward `tensor_tensor_scan` over all line
blocks concatenated in the free dim, with 256-wide INF separator regions
between blocks (a cross-block leak path costs >= 256 > max(d) = 255, so
leaks never win a min). Diagonal passes run in 45-degree-sheared layouts
produced by DRAM staging buffers with mismatched read/write row pitches;
reads come back through 16-bit DMA-transposes straight into the scan
layout. S(p) uses PE banded matmuls for row-shifted d and DVE is_lt
masks.

Data-parallel over B*C = 2 images: core b computes image b.
"""

import os
import numpy as np

import concourse.bacc as bacc
import concourse.mybir as mybir
from concourse.tile import TileContext
from concourse.masks import make_identity
from concourse.bass_utils import run_bass_kernel_spmd

F32 = mybir.dt.float32
F16 = mybir.dt.float16
I16 = mybir.dt.int16
AF = mybir.ActivationFunctionType
ALU = mybir.AluOpType

H = W = 256
HB = 2
INF = 1536.0
H_PARAM = np.float32(0.35)
E1 = float(np.exp(np.float32(-1.0) / H_PARAM))
EC = float(np.exp(np.float32(-np.sqrt(np.float32(2.0))) / H_PARAM))
LNSCALE = float(np.exp(np.float32(1.0) / H_PARAM))

P1R = 516   # stage1 read pitch (f16); write pitch 515, base 255: c = x+255-y
P2R = 768   # stage2 read pitch (f16); write pitch 770: c' = c+2y-255
P3R = 516   # stage3 read pitch (f16); write pitch 515: x = c'-y

N_CORES = 8


def _build_program():
    nc = bacc.Bacc("TRN2", target_bir_lowering=False, debug=False,
                   num_devices=N_CORES)
    img = nc.dram_tensor("img", [H, W], F32, kind="ExternalInput").ap()
    out = nc.dram_tensor("out", [H, W], F32, kind="ExternalOutput").ap()
    stage1 = nc.dram_tensor("stage1", [256 * P1R + 600], F16).ap()
    stage2 = nc.dram_tensor("stage2", [256 * P2R + 1200], F16).ap()
    stage3 = nc.dram_tensor("stage3", [256 * P3R + 600], F16).ap()

    dbg = {}
    if os.environ.get("DT_DEBUG"):
        for name, shape in [("dbg_d", [H, W]), ("dbg_dstar", [H, W]),
                            ("dbg_s", [H, W])]:
            dbg[name] = nc.dram_tensor(name, shape, F32,
                                       kind="ExternalOutput").ap()

    with TileContext(nc) as tc:
        _emit(nc, tc, img, out, stage1, stage2, stage3, dbg)
    nc.compile()
    return nc


def _emit(nc, tc, img, out, stage1, stage2, stage3, dbg=None):
    import contextlib
    dbg = dbg or {}
    ctx = contextlib.ExitStack()
    const = ctx.enter_context(tc.tile_pool(name="const", bufs=1))
    work = ctx.enter_context(tc.tile_pool(name="work", bufs=1))
    psum = ctx.enter_context(tc.tile_pool(name="psum", bufs=4, space="PSUM"))
    psum2 = ctx.enter_context(tc.tile_pool(name="psum2", bufs=2, space="PSUM"))

    # ---------------- critical-path head ----------------------------------
    inf16 = const.tile([128, 1540], F16)
    nc.vector.memset(inf16[:], INF)

    # load the seed image (also used by the final stage)
    img32 = work.tile([128, HB, 256], F32)
    nc.sync.dma_start(out=img32[:, 0, :], in_=img[0:128, :])
    nc.scalar.dma_start(out=img32[:, 1, :], in_=img[128:256, :])

    # prefill stage1 with INF (field is post-transform: seeds 0, empty INF)
    n1 = 256 * P1R
    nc.sync.dma_start(
        out=stage1[:n1].rearrange("(p f) -> p f", p=128),
        in_=inf16[:, : n1 // 128])
    # prefill stage2 with INF
    n2 = 256 * P2R
    nc.scalar.dma_start(
        out=stage2[:n2].rearrange("(p f) -> p f", p=128),
        in_=inf16[:, : n2 // 128])

    # seed transform on-chip: 0 -> INF, 1 -> 0 (fp16)
    d0 = work.tile([128, HB, 256], F16)
    nc.scalar.activation(d0[:], img32[:], AF.Copy, bias=INF, scale=-INF)
    # sheared band write: row y at 255 + 515*y + x; read c = x+255-y
    bandA = stage1[255: 255 + 256 * (P1R - 1)].rearrange(
        "(y f) -> y f", f=P1R - 1)[:, :W]
    nc.sync.dma_start(out=bandA[0:128, :], in_=d0[:, 0, :])
    nc.scalar.dma_start(out=bandA[128:256, :], in_=d0[:, 1, :])

    # ---------------- constants / init ------------------------------------
    ident16 = const.tile([128, 128], F16)
    make_identity(nc, ident16[:])

    ones16 = const.tile([128, 256], F16)
    nc.gpsimd.memset(ones16[:], 1.0)

    zero32 = const.tile([128, 512], F32)
    nc.gpsimd.memset(zero32[:], 0.0)

    # sanitize mask (transposed layout): MBT[p, cb, y] = 1 where
    # c' = 128*cb + p is outside [y, y+255]. For cb in {0,1} only c'-y < 0
    # can be invalid; for cb in {2,3} only c'-y > 255.
    VT = const.tile([128, 4, 256], F16)
    nc.vector.memset(VT[:], 1.0)
    for cb in range(4):
        if cb < 2:
            nc.gpsimd.affine_select(   # valid iff (128*cb + p) - y >= 0
                out=VT[:, cb, :], in_=VT[:, cb, :], compare_op=ALU.is_ge,
                fill=0.0, base=128 * cb, pattern=[[-1, 256]],
                channel_multiplier=1)
        else:
            nc.gpsimd.affine_select(   # valid iff 255 - (128*cb + p) + y >= 0
                out=VT[:, cb, :], in_=VT[:, cb, :], compare_op=ALU.is_ge,
                fill=0.0, base=255 - 128 * cb, pattern=[[1, 256]],
                channel_multiplier=-1)
    FILLT = const.tile([128, 4, 256], F16)   # (1 - V) * INF
    nc.scalar.activation(FILLT[:], VT[:], AF.Copy, bias=INF, scale=-INF)

    # banded matrices for row shifts (lhsT: [k, m] = weight of in-row k in
    # out-row m). up: out[m] = in[m-1] (replicate top); down: out[m]=in[m+1].
    def band(tile_ap, diag_base, corner=None):
        nc.gpsimd.memset(tile_ap, 0.0)
        nc.gpsimd.affine_select(
            out=tile_ap, in_=tile_ap, compare_op=ALU.not_equal, fill=1.0,
            base=diag_base, pattern=[[-1, 128]], channel_multiplier=1)
        if corner == "tl":
            nc.gpsimd.affine_select(
                out=tile_ap, in_=tile_ap, compare_op=ALU.not_equal, fill=1.0,
                base=0, pattern=[[1, 128]], channel_multiplier=1)
        elif corner == "br":
            nc.gpsimd.affine_select(
                out=tile_ap, in_=tile_ap, compare_op=ALU.not_equal, fill=1.0,
                base=-254, pattern=[[1, 128]], channel_multiplier=1)

    supA = const.tile([128, 128], F16)
    band(supA[:], 1, corner="tl")
    supB = const.tile([128, 128], F16)
    band(supB[:], 1)
    sdnA = const.tile([128, 128], F16)
    band(sdnA[:], -1)
    sdnB = const.tile([128, 128], F16)
    band(sdnB[:], -1, corner="br")
    e_up = const.tile([128, 128], F16)   # 1 at [k=127, m=0]
    nc.gpsimd.memset(e_up[:], 0.0)
    nc.gpsimd.affine_select(
        out=e_up[:], in_=e_up[:], compare_op=ALU.not_equal, fill=1.0,
        base=127, pattern=[[1, 128]], channel_multiplier=-1)
    e_dn = const.tile([128, 128], F16)   # 1 at [k=0, m=127]
    nc.gpsimd.memset(e_dn[:], 0.0)
    nc.gpsimd.affine_select(
        out=e_dn[:], in_=e_dn[:], compare_op=ALU.not_equal, fill=1.0,
        base=127, pattern=[[-1, 128]], channel_multiplier=1)

    inv32 = work.tile([128, HB, 256], F32)   # 1 - seed
    nc.scalar.activation(inv32[:], img32[:], AF.Copy, bias=1.0, scale=-1.0)

    # ---------------- shear A + transpose-in ------------------------------
    sk1t = work.tile([128, HB, 512], F16)
    for b in range(HB):
        rd = stage1[128 * b * P1R: (128 * b + 128) * P1R].rearrange(
            "(y f) -> y f", f=P1R)[:, :512]
        eng = nc.sync if b == 0 else nc.scalar
        eng.dma_start(out=sk1t[:, b, :], in_=rd)

    def transpose_2to4(srct, dstt):
        k = 0
        for cb in range(4):
            for yb in range(2):
                pt = psum.tile([128, 128], F16, tag="tp")
                nc.tensor.transpose(
                    pt[:], srct[:, yb, 128 * cb:128 * (cb + 1)], ident16[:])
                if k % 2 == 0:
                    nc.scalar.copy(out=dstt[:, cb, 128 * yb:128 * (yb + 1)],
                                   in_=pt[:])
                else:
                    nc.vector.tensor_copy(
                        out=dstt[:, cb, 128 * yb:128 * (yb + 1)], in_=pt[:])
                k += 1

    d1 = work.tile([128, 4, 256], F16)
    transpose_2to4(sk1t, d1)

    # ---------------- per-block min-plus pass helper ------------------------
    def minplus(arr, tmp_tag):
        nblk = arr.shape[1]
        for cb in range(nblk):
            tmp = work.tile([128, 256], F16, tag=tmp_tag)
            nc.vector.tensor_tensor_scan(
                out=tmp[:], data0=ones16[:], data1=arr[:, cb, :],
                initial=INF, op0=ALU.add, op1=ALU.min)
            nc.vector.tensor_tensor_scan(
                out=arr[:, cb, ::-1], data0=ones16[:],
                data1=tmp[:, ::-1], initial=INF, op0=ALU.add, op1=ALU.min)

    minplus(d1, "scan1")          # diag SE pass (lines c = x-y+255)

    # ---------------- transpose-back + shear B -----------------------------
    def transpose_4to2(src, dst):
        k = 0
        for yb in range(2):
            for cb in range(4):
                pt = psum.tile([128, 128], F16, tag="tp")
                nc.tensor.transpose(
                    pt[:], src[:, cb, 128 * yb:128 * (yb + 1)], ident16[:])
                if k % 2 == 0:
                    nc.scalar.copy(out=dst[:, yb, 128 * cb:128 * (cb + 1)],
                                   in_=pt[:])
                else:
                    nc.vector.tensor_copy(
                        out=dst[:, yb, 128 * cb:128 * (cb + 1)], in_=pt[:])
                k += 1

    sk1b = work.tile([128, HB, 512], F16)
    transpose_4to2(d1, sk1b)

    # write(y, c) at 770*y + c ; read(y, c') at 768*y + 255 + c'
    for b in range(HB):
        wr = stage2[128 * b * (P2R + 2): (128 * b + 128) * (P2R + 2)].rearrange(
            "(y f) -> y f", f=P2R + 2)[:, :512]
        nc.sync.dma_start(out=wr, in_=sk1b[:, b, :])
    sk2 = work.tile([128, HB, 512], F16)
    for b in range(HB):
        rd = stage2[255 + 128 * b * P2R: 255 + (128 * b + 128) * P2R].rearrange(
            "(y f) -> y f", f=P2R)[:, :512]
        eng = nc.sync if b == 0 else nc.scalar
        eng.dma_start(out=sk2[:, b, :], in_=rd)
    d2 = work.tile([128, 4, 256], F16)
    transpose_2to4(sk2, d2)
    nc.vector.tensor_tensor(out=d2[:], in0=d2[:], in1=VT[:], op=ALU.mult)
    nc.vector.tensor_tensor(out=d2[:], in0=d2[:], in1=FILLT[:], op=ALU.add)

    minplus(d2, "scan2")          # diag NE pass (lines c' = x+y)

    sk2b = work.tile([128, HB, 512], F16)
    transpose_4to2(d2, sk2b)

    # ---------------- unshear C -------------------------------------------
    # write(y, c') at 515*y + c' ; read(y, x) at 516*y + x  (x = c'-y)
    for b in range(HB):
        wr = stage3[128 * b * (P3R - 1): (128 * b + 128) * (P3R - 1)].rearrange(
            "(y f) -> y f", f=P3R - 1)[:, :512]
        nc.sync.dma_start(out=wr, in_=sk2b[:, b, :])
    dstar = work.tile([128, HB, 256], F16)
    for b in range(HB):
        rd = stage3[128 * b * P3R: (128 * b + 128) * P3R].rearrange(
            "(y f) -> y f", f=P3R)[:, :256]
        nc.sync.dma_start(out=dstar[:, b, :], in_=rd)
    dstT = work.tile([128, HB, 256], F16)
    k = 0
    for xb in range(2):
        for yb in range(2):
            pt = psum.tile([128, 128], F16, tag="tp")
            nc.tensor.transpose(
                pt[:], dstar[:, yb, 128 * xb:128 * (xb + 1)], ident16[:])
            if k % 2 == 0:
                nc.scalar.copy(out=dstT[:, xb, 128 * yb:128 * (yb + 1)],
                               in_=pt[:])
            else:
                nc.vector.tensor_copy(
                    out=dstT[:, xb, 128 * yb:128 * (yb + 1)], in_=pt[:])
            k += 1

    if "dbg_dstar" in dbg:
        t = work.tile([128, HB, 256], F32, tag="dbg1")
        nc.vector.tensor_copy(out=t[:], in_=dstar[:])
        for b in range(HB):
            nc.sync.dma_start(out=dbg["dbg_dstar"][128 * b:128 * (b + 1), :],
                              in_=t[:, b, :])

    # ---------------- axis passes ------------------------------------------
    minplus(dstar, "scan3")       # axisX in image layout
    minplus(dstT, "scan4")        # axisY in transposed layout
    dy = work.tile([128, HB, 256], F16)
    k = 0
    for yb in range(2):
        for xb in range(2):
            pt = psum.tile([128, 128], F16, tag="tp")
            nc.tensor.transpose(
                pt[:], dstT[:, xb, 128 * yb:128 * (yb + 1)], ident16[:])
            if k % 2 == 0:
                nc.scalar.copy(out=dy[:, yb, 128 * xb:128 * (xb + 1)],
                               in_=pt[:])
            else:
                nc.vector.tensor_copy(out=dy[:, yb, 128 * xb:128 * (xb + 1)],
                                      in_=pt[:])
            k += 1

    d16 = work.tile([128, HB, 256], F16)
    nc.vector.tensor_tensor(out=d16[:], in0=dstar[:], in1=dy[:], op=ALU.min)

    if "dbg_d" in dbg:
        t = work.tile([128, HB, 256], F32, tag="dbg2")
        nc.vector.tensor_copy(out=t[:], in_=d16[:])
        for b in range(HB):
            nc.sync.dma_start(out=dbg["dbg_d"][128 * b:128 * (b + 1), :],
                              in_=t[:, b, :])

    # ---------------- S stage ---------------------------------------------
    pup = psum2.tile([128, HB, 256], F32, tag="pup")
    nc.tensor.matmul(pup[:, 0, :], supA[:], d16[:, 0, :], start=True, stop=True)
    nc.tensor.matmul(pup[:, 1, :], supB[:], d16[:, 1, :], start=True, stop=False)
    nc.tensor.matmul(pup[:, 1, :], e_up[:], d16[:, 0, :], start=False, stop=True)
    pdn = psum2.tile([128, HB, 256], F32, tag="pdn")
    nc.tensor.matmul(pdn[:, 0, :], sdnA[:], d16[:, 0, :], start=True, stop=False)
    nc.tensor.matmul(pdn[:, 0, :], e_dn[:], d16[:, 1, :], start=False, stop=True)
    nc.tensor.matmul(pdn[:, 1, :], sdnB[:], d16[:, 1, :], start=True, stop=True)
    up16 = work.tile([128, HB, 256], F16)
    nc.scalar.copy(out=up16[:], in_=pup[:])
    dn16 = work.tile([128, HB, 256], F16)
    nc.scalar.copy(out=dn16[:], in_=pdn[:])

    GA = work.tile([128, 4, HB, 256], F16)
    GD = work.tile([128, 4, HB, 256], F16)
    for b in range(HB):   # zero only the never-written border columns
        nc.gpsimd.memset(GA[:, 2, b, 0:1], 0.0)
        nc.gpsimd.memset(GA[:, 3, b, 255:256], 0.0)

    nc.vector.tensor_tensor(out=GA[:, 0], in0=up16[:], in1=d16[:], op=ALU.is_lt)
    nc.vector.tensor_tensor(out=GA[:, 1], in0=dn16[:], in1=d16[:], op=ALU.is_lt)
    for b in range(HB):
        nc.vector.tensor_tensor(
            out=GA[:, 2, b, 1:], in0=d16[:, b, :-1], in1=d16[:, b, 1:],
            op=ALU.is_lt)
        nc.vector.tensor_tensor(
            out=GA[:, 3, b, :-1], in0=d16[:, b, 1:], in1=d16[:, b, :-1],
            op=ALU.is_lt)
        nc.vector.tensor_tensor(
            out=GD[:, 0, b, 1:], in0=up16[:, b, :-1], in1=d16[:, b, 1:],
            op=ALU.is_lt)
        nc.vector.tensor_tensor(
            out=GD[:, 1, b, :-1], in0=up16[:, b, 1:], in1=d16[:, b, :-1],
            op=ALU.is_lt)
        nc.vector.tensor_tensor(
            out=GD[:, 2, b, 1:], in0=dn16[:, b, :-1], in1=d16[:, b, 1:],
            op=ALU.is_lt)
        nc.vector.tensor_tensor(
            out=GD[:, 3, b, :-1], in0=dn16[:, b, 1:], in1=d16[:, b, :-1],
            op=ALU.is_lt)
    for b in range(HB):   # x-border clamp: diagonals collapse onto verticals
        nc.scalar.copy(out=GD[:, 0, b, 0:1], in_=GA[:, 0, b, 0:1])
        nc.scalar.copy(out=GD[:, 2, b, 0:1], in_=GA[:, 1, b, 0:1])
        nc.scalar.copy(out=GD[:, 1, b, 255:256], in_=GA[:, 0, b, 255:256])
        nc.scalar.copy(out=GD[:, 3, b, 255:256], in_=GA[:, 1, b, 255:256])

    sa01 = work.tile([128, HB, 256], F16)
    nc.vector.tensor_tensor(out=sa01[:], in0=GA[:, 0], in1=GA[:, 1], op=ALU.add)
    sa23 = work.tile([128, HB, 256], F16)
    nc.vector.tensor_tensor(out=sa23[:], in0=GA[:, 2], in1=GA[:, 3], op=ALU.add)
    SA = work.tile([128, HB, 256], F16)
    nc.vector.tensor_tensor(out=SA[:], in0=sa01[:], in1=sa23[:], op=ALU.add)
    sd01 = work.tile([128, HB, 256], F16)
    nc.vector.tensor_tensor(out=sd01[:], in0=GD[:, 0], in1=GD[:, 1], op=ALU.add)
    sd23 = work.tile([128, HB, 256], F16)
    nc.vector.tensor_tensor(out=sd23[:], in0=GD[:, 2], in1=GD[:, 3], op=ALU.add)
    SD = work.tile([128, HB, 256], F16)
    nc.vector.tensor_tensor(out=SD[:], in0=sd01[:], in1=sd23[:], op=ALU.add)

    sa32 = work.tile([128, HB, 256], F32)
    nc.scalar.activation(sa32[:], SA[:], AF.Copy, bias=0.0, scale=E1)
    s32 = work.tile([128, HB, 256], F32)
    nc.vector.scalar_tensor_tensor(
        out=s32[:], in0=SD[:], scalar=EC, in1=sa32[:],
        op0=ALU.mult, op1=ALU.add)

    if "dbg_s" in dbg:
        for b in range(HB):
            nc.sync.dma_start(out=dbg["dbg_s"][128 * b:128 * (b + 1), :],
                              in_=s32[:, b, :])

    sg = work.tile([128, HB, 256], F32)
    nc.vector.tensor_tensor(out=sg[:], in0=s32[:], in1=img32[:], op=ALU.add)
    lnv = work.tile([128, HB, 256], F32)
    nc.scalar.activation(lnv[:], sg[:], AF.Ln, bias=0.0, scale=LNSCALE)
    outp = work.tile([128, HB, 256], F32)
    nc.vector.scalar_tensor_tensor(
        out=outp[:], in0=lnv[:], scalar=float(-H_PARAM), in1=d16[:],
        op0=ALU.mult, op1=ALU.add)
    nc.vector.tensor_tensor(out=outp[:], in0=outp[:], in1=inv32[:],
                            op=ALU.mult)

    for b in range(HB):
        nc.sync.dma_start(out=out[128 * b:128 * (b + 1), :], in_=outp[:, b, :])

    ctx.close()


_NC_CACHE = None


def _get_nc():
    global _NC_CACHE
    if _NC_CACHE is None:
        _NC_CACHE = _build_program()
    return _NC_CACHE


def kernel(image: np.ndarray) -> np.ndarray:
    """image: (2, 1, 256, 256) float32 -> (2, 1, 256, 256) float32."""
    B, C, Himg, Wimg = image.shape
    flat = np.ascontiguousarray(
        image.reshape(B * C, Himg, Wimg).astype(np.float32))
    n_units = flat.shape[0]
    nc = _get_nc()
    in_maps = [{"img": flat[i % n_units]} for i in range(N_CORES)]
    res = run_bass_kernel_spmd(nc, in_maps, core_ids=list(range(N_CORES)))
    outs = [res.results[i]["out"] for i in range(n_units)]
    return np.stack(outs).reshape(B, C, Himg, Wimg).astype(image.dtype)


if __name__ == "__main__":
    from concourse.bass_interp import CoreSim
    import jax
    cpu = jax.devices("cpu")[0]
    with jax.default_device(cpu):
        import reference as R
        inputs = R.setup_inputs()
        img_np = np.asarray(inputs["image"]).reshape(2, 256, 256)
        expected = np.asarray(R.reference(**inputs)).reshape(2, 256, 256)
    print("reference done", flush=True)
    nc = _get_nc()
    print("program built", flush=True)
    sim = CoreSim(nc)
    sim.tensor("img")[:] = img_np[0]
    sim.simulate()
    got = sim.tensor("out").copy()
    err = np.abs(got - expected[0])
    rel = err.max() / (np.abs(expected[0]).max() + 1e-9)
    print("sim image0: max abs err", err.max(), "rel", rel)
